# revision 13
# baseline (speedup 1.0000x reference)
"""Trainium2 Bass kernel for AttnPainterOilDensity (per-pixel top-10 stroke
selection + back-to-front alpha compositing).

Math (per pixel, strokes n = 0..255):
  m_n   = alpha_n > 0.1
  E_n   = #{k > n : m_k}                      (visible strokes in front)
  sel_n = m_n and E_n <= 9                    (the last 10 visible strokes)
  ae_n  = alpha_n * sel_n
  lg_n  = ln(1 - ae_n)                        (0 for unselected)
  Lx_n  = sum_{k>n} lg_k ;  Texcl_n = exp(Lx_n)
  w_n   = ae_n * Texcl_n                      (compositing weight)
  canvas_c = sum_n w_n * color_{n,c} + exp(sum_n lg_n)
  den      = sum_n w_n * s_n        + exp(sum_n lg_n),  s_n = p2_n * p3_n

Layout: stroke-major tiles [128 strokes, 512 pixels]; the per-pixel suffix
counts/sums over the stroke (partition) axis run on the PE via triangular
constant stationaries; selection via one fused scalar_tensor_tensor; ln/exp
on ACT; per-stroke reductions back to per-pixel rows via small matmuls.

Sharding: 8 cores = (batch b = core//2) x (half of the 128x128 plane).
"""

import numpy as np
import ml_dtypes

import concourse.bacc as bacc
import concourse.bass as bass
import concourse.tile as tile
from concourse import mybir
from concourse.bass_utils import run_bass_kernel_spmd


def _patch_act_tables():
    # Force Ln and Exp onto the shared natural_log_exp_and_others set so the
    # per-tile Ln -> Exp alternation doesn't reload ACT tables 2x per tile.
    if _CACHED.get("act_patched"):
        return
    import concourse.hw_specs as hw_specs
    orig = hw_specs.get_activation_tables

    def patched(arch):
        tables = dict(orig(arch))
        ln = mybir.ActivationFunctionType.Ln
        ex = mybir.ActivationFunctionType.Exp
        for name, fns in tables.items():
            if name != "natural_log_exp_and_others":
                tables[name] = fns - {ln, ex}
        return tables

    hw_specs.get_activation_tables = patched
    bacc.get_activation_tables = patched
    _CACHED["act_patched"] = True

B, N, H, W = 4, 256, 128, 128
PIX = H * W // 2          # pixels per core (half plane) = 8192
F = 512                   # pixels per tile
NT = PIX // F             # 16 tiles
BIG = 1024.0
THRESH = -1014.5          # q <= 9 - BIG

f32 = mybir.dt.float32
f32r = mybir.dt.float32r
bf16 = mybir.dt.bfloat16

_CACHED = {}


def _build_program():
    _patch_act_tables()
    nc = bacc.Bacc("TRN2", target_bir_lowering=False, debug=False, num_devices=8)

    a_d = nc.dram_tensor("alpha", [N, PIX], f32, kind="ExternalInput")
    c_d = nc.dram_tensor("color", [N, 3, PIX], f32, kind="ExternalInput")
    p_d = nc.dram_tensor("par", [N, 8], f32, kind="ExternalInput")
    u2b_d = nc.dram_tensor("U2B", [128, 128], bf16, kind="ExternalInput")
    onb_d = nc.dram_tensor("ONESB", [128, 128], bf16, kind="ExternalInput")
    thr_d = nc.dram_tensor("THR", [128, 1], f32, kind="ExternalInput")
    p64_d = nc.dram_tensor("P64", [1, 128], bf16, kind="ExternalInput")
    neg01_d = nc.dram_tensor("NEG01", [128, 1], f32, kind="ExternalInput")
    usr_d = nc.dram_tensor("USR", [128, 128], f32r, kind="ExternalInput")
    onr_d = nc.dram_tensor("ONESR", [128, 128], f32r, kind="ExternalInput")
    stc_d = nc.dram_tensor("STC", [128, 4, 8], f32r, kind="ExternalInput")
    bgw_d = nc.dram_tensor("BGW", [1, 8], f32r, kind="ExternalInput")
    z8_d = nc.dram_tensor("Z8", [128, 8], f32r, kind="ExternalInput")
    out_d = nc.dram_tensor("out", [4, PIX], f32, kind="ExternalOutput")

    Ln = mybir.ActivationFunctionType.Ln
    Exp = mybir.ActivationFunctionType.Exp
    Sign = mybir.ActivationFunctionType.Sign
    MUL = mybir.AluOpType.mult
    LE = mybir.AluOpType.is_le
    GT = mybir.AluOpType.is_gt

    with tile.TileContext(nc) as tc:
        with (
            tc.tile_pool(name="cst", bufs=1) as cst,
            tc.tile_pool(name="io", bufs=5) as io,
            tc.tile_pool(name="wk", bufs=3) as wk,
            tc.tile_pool(name="psq", bufs=2, space="PSUM") as psq,
            tc.tile_pool(name="psl", bufs=1, space="PSUM") as psl,
            tc.tile_pool(name="psr", bufs=2, space="PSUM") as psr,
        ):
            # ---- constants / per-core setup ----
            hp = tc.high_priority(offset=100000)
            hp.__enter__()
            u2b = cst.tile([128, 128], bf16)
            nc.sync.dma_start(u2b[:], u2b_d[:])
            onb = cst.tile([128, 128], bf16)
            nc.sync.dma_start(onb[:], onb_d[:])
            usr = cst.tile([128, 128], f32r)
            nc.sync.dma_start(usr[:], usr_d[:])
            onr = cst.tile([128, 128], f32r)
            nc.sync.dma_start(onr[:], onr_d[:])
            stc = cst.tile([128, 4, 8], f32r)
            nc.sync.dma_start(stc[:], stc_d[:])
            bgw = cst.tile([1, 8], f32r)
            nc.sync.dma_start(bgw[:], bgw_d[:])
            thr = cst.tile([128, 1], f32)
            nc.sync.dma_start(thr[:], thr_d[:])
            p64 = cst.tile([1, 128], bf16)
            nc.sync.dma_start(p64[:], p64_d[:])
            onerow = cst.tile([1, F], bf16)
            nc.vector.memset(onerow[:], 1.0)
            neg01 = cst.tile([128, 1], f32)
            nc.sync.dma_start(neg01[:], neg01_d[:])

            par0 = cst.tile([128, 8], f32)
            nc.sync.dma_start(par0[:], p_d[0:128, :])
            par1 = cst.tile([128, 8], f32)
            nc.sync.dma_start(par1[:], p_d[128:256, :])
            st3l = cst.tile([128, 8], f32r)
            nc.sync.dma_start(st3l[:], z8_d[:])
            st3h = cst.tile([128, 8], f32r)
            nc.sync.dma_start(st3h[:], z8_d[:])
            nc.vector.tensor_tensor(st3l[:, 4:5], par0[:, 2:3], par0[:, 3:4], MUL)
            nc.vector.tensor_tensor(st3h[:, 4:5], par1[:, 2:3], par1[:, 3:4], MUL)
            hp.__exit__(None, None, None)

            # ---- main loop over pixel tiles ----
            for t in range(NT):
                px = bass.ts(t, F)

                At = io.tile([128, 2, F], f32, tag="A")
                Ct = io.tile([128, 2, 3, F], f32, tag="C")
                with tc.high_priority(offset=60):
                    nc.sync.dma_start(
                        At[:],
                        bass.AP(a_d, t * F, [[PIX, 128], [128 * PIX, 2], [1, F]]),
                    )
                    for h in range(2):
                        nc.sync.dma_start(
                            Ct[:, h],
                            bass.AP(
                                c_d, h * 128 * 3 * PIX + t * F,
                                [[3 * PIX, 128], [PIX, 3], [1, F]],
                            ),
                        )

                # msign = sign(A - 0.1) in {-1,+1}; mask algebra folded into
                # halved stationaries, per-row thresholds and a +64 rank-1.
                ms = wk.tile([128, 2, F], bf16, tag="ms")
                nc.scalar.activation(ms[:], At[:], Sign, bias=neg01[:])

                q2 = psq.tile([128, 2, F], f32, tag="q2")
                nc.tensor.matmul(q2[:, 1, :], u2b[:], ms[:, 1, :], start=True, stop=True)
                nc.tensor.matmul(q2[:, 0, :], u2b[:], ms[:, 0, :], start=True, stop=False)
                nc.tensor.matmul(q2[:, 0, :], onb[:], ms[:, 1, :], start=False, stop=False)
                nc.tensor.matmul(q2[:, 0, :], p64[:], onerow[:], start=False, stop=True)

                # ae = (q <= thr) * A over both halves at once
                aet = wk.tile([128, 2, F], f32, tag="aet")
                nc.vector.scalar_tensor_tensor(aet[:], q2[:], thr[:], At[:], LE, MUL)

                # lg = ln(1 - ae)
                lgt = wk.tile([128, 2, F], f32r, tag="lgt")
                nc.scalar.activation(lgt[:], aet[:], Ln, bias=1.0, scale=-1.0)

                # suffix log-sums
                Lx = psl.tile([128, 2, F], f32, tag="Lx")
                nc.tensor.matmul(Lx[:, 1, :], usr[:], lgt[:, 1, :], start=True, stop=True)
                nc.tensor.matmul(Lx[:, 0, :], usr[:], lgt[:, 0, :], start=True, stop=False)
                nc.tensor.matmul(Lx[:, 0, :], onr[:], lgt[:, 1, :], start=False, stop=True)

                Txt = wk.tile([128, 2, F], f32, tag="Txt")
                nc.scalar.activation(Txt[:], Lx[:], Exp)

                # w = ae * Texcl
                wt = wk.tile([128, 2, 1, F], f32r, tag="wt")
                nc.vector.tensor_tensor(wt[:, :, 0, :], aet[:], Txt[:], MUL)

                # Z = w (broadcast over channel) * color, one op
                Zt = wk.tile([128, 2, 3, F], f32r, tag="Zt")
                nc.vector.tensor_tensor(
                    Zt[:], wt[:].to_broadcast([128, 2, 3, F]), Ct[:], MUL
                )

                # reductions: row0 sum(lg), rows 1-3 canvas rgb, row 4 den
                red = psr.tile([8, F], f32, tag="red")
                nc.tensor.matmul(red[:], stc[:, 0, :], Zt[:, 1, 0, :], start=True, stop=False)
                nc.tensor.matmul(red[:], stc[:, 0, :], Zt[:, 0, 0, :], start=False, stop=False)
                nc.tensor.matmul(red[:], stc[:, 1, :], Zt[:, 1, 1, :], start=False, stop=False)
                nc.tensor.matmul(red[:], stc[:, 1, :], Zt[:, 0, 1, :], start=False, stop=False)
                nc.tensor.matmul(red[:], stc[:, 2, :], Zt[:, 1, 2, :], start=False, stop=False)
                nc.tensor.matmul(red[:], stc[:, 2, :], Zt[:, 0, 2, :], start=False, stop=False)
                nc.tensor.matmul(red[:], st3h[:], wt[:, 1, 0, :], start=False, stop=False)
                nc.tensor.matmul(red[:], st3l[:], wt[:, 0, 0, :], start=False, stop=False)
                nc.tensor.matmul(red[:], stc[:, 3, :], lgt[:, 1, :], start=False, stop=False)
                nc.tensor.matmul(red[:], stc[:, 3, :], lgt[:, 0, :], start=False, stop=False)

                # background transmittance onto rows 1-4 via rank-1 matmul
                bg = wk.tile([1, F], f32r, tag="bg")
                nc.scalar.activation(bg[:], red[0:1, :], Exp)
                nc.tensor.matmul(red[:], bgw[:], bg[:], start=False, stop=True)

                if t % 4 == 0:
                    outt = wk.tile([8, 4, F], f32, tag="outt")
                nc.scalar.copy(outt[:, t % 4, :], red[:])
                if t % 4 == 3:
                    nc.sync.dma_start(
                        bass.AP(out_d, (t - 3) * F, [[PIX, 4], [1, 4 * F]]),
                        outt[1:5, :, :]
                    )

    nc.compile()
    return nc


def _get_program():
    if "nc" not in _CACHED:
        _CACHED["nc"] = _build_program()
    return _CACHED["nc"]


def _consts():
    if "consts" in _CACHED:
        return _CACHED["consts"]
    tri = np.tril(np.ones((128, 128), np.float32), -1)
    # halved: the matmul consumes msign in {-1,+1}; q = U2h@msign + r
    u2 = 0.5 * tri - (BIG / 2) * np.eye(128, dtype=np.float32)
    ones = np.ones((128, 128), np.float32)
    r = u2.sum(axis=0).astype(np.float32).reshape(128, 1)   # per-row offset
    thr = np.float32(THRESH) - r
    stc = np.zeros((128, 4, 8), np.float32)
    stc[:, 0, 1] = 1.0   # canvas r -> row 1
    stc[:, 1, 2] = 1.0   # canvas g -> row 2
    stc[:, 2, 3] = 1.0   # canvas b -> row 3
    stc[:, 3, 0] = 1.0   # sum(lg) -> row 0
    bgw = np.zeros((1, 8), np.float32)
    bgw[0, 1:5] = 1.0
    consts = {
        "U2B": u2.astype(ml_dtypes.bfloat16),
        "ONESB": (0.5 * ones).astype(ml_dtypes.bfloat16),
        "THR": thr,
        "P64": np.full((1, 128), 64.0, ml_dtypes.bfloat16),
        "NEG01": np.full((128, 1), -np.float32(0.1), np.float32),
        "USR": tri,
        "ONESR": ones,
        "STC": stc,
        "BGW": bgw,
        "Z8": np.zeros((128, 8), np.float32),
    }
    _CACHED["consts"] = consts
    return consts


def _make_in_maps(color_stroke, alpha, params):
    consts = _consts()
    in_maps = []
    for core in range(8):
        b, half = core // 2, core % 2
        r0 = half * (H // 2)
        a = np.ascontiguousarray(
            alpha[b, :, 0, r0 : r0 + H // 2, :].reshape(N, PIX)
        )
        c = np.ascontiguousarray(
            color_stroke[b, :, :, r0 : r0 + H // 2, :].reshape(N, 3, PIX)
        )
        p = np.ascontiguousarray(params[b])
        in_maps.append({"alpha": a, "color": c, "par": p, **consts})
    return in_maps


def kernel(color_stroke, alpha, params, _trace=False, _trace_kwargs=None):
    color_stroke = np.asarray(color_stroke, dtype=np.float32)
    alpha = np.asarray(alpha, dtype=np.float32)
    params = np.asarray(params, dtype=np.float32)

    nc = _get_program()
    in_maps = _make_in_maps(color_stroke, alpha, params)
    res = run_bass_kernel_spmd(
        nc, in_maps, list(range(8)), trace=_trace, **(_trace_kwargs or {})
    )
    _CACHED["last_result"] = res

    canvas = np.empty((B, 3, H, W), np.float32)
    den = np.empty((B, 1, H, W), np.float32)
    for core in range(8):
        b, half = core // 2, core % 2
        r0 = half * (H // 2)
        o = res.results[core]["out"]
        canvas[b, :, r0 : r0 + H // 2, :] = o[0:3].reshape(3, H // 2, W)
        den[b, 0, r0 : r0 + H // 2, :] = o[3].reshape(H // 2, W)
    return canvas, den


# revision 14
# speedup vs baseline: 1.0157x; 1.0157x over previous
"""Trainium2 Bass kernel for AttnPainterOilDensity (per-pixel top-10 stroke
selection + back-to-front alpha compositing).

Math (per pixel, strokes n = 0..255):
  m_n   = alpha_n > 0.1
  E_n   = #{k > n : m_k}                      (visible strokes in front)
  sel_n = m_n and E_n <= 9                    (the last 10 visible strokes)
  ae_n  = alpha_n * sel_n
  lg_n  = ln(1 - ae_n)                        (0 for unselected)
  Lx_n  = sum_{k>n} lg_k ;  Texcl_n = exp(Lx_n)
  w_n   = ae_n * Texcl_n                      (compositing weight)
  canvas_c = sum_n w_n * color_{n,c} + exp(sum_n lg_n)
  den      = sum_n w_n * s_n        + exp(sum_n lg_n),  s_n = p2_n * p3_n

Layout: stroke-major tiles [128 strokes, 512 pixels]; the per-pixel suffix
counts/sums over the stroke (partition) axis run on the PE via triangular
constant stationaries; selection via one fused scalar_tensor_tensor; ln/exp
on ACT; per-stroke reductions back to per-pixel rows via small matmuls.

Sharding: 8 cores = (batch b = core//2) x (half of the 128x128 plane).
"""

import numpy as np
import ml_dtypes

import concourse.bacc as bacc
import concourse.bass as bass
import concourse.tile as tile
from concourse import mybir
from concourse.bass_utils import run_bass_kernel_spmd


def _patch_act_tables():
    # Force Ln and Exp onto the shared natural_log_exp_and_others set so the
    # per-tile Ln -> Exp alternation doesn't reload ACT tables 2x per tile.
    if _CACHED.get("act_patched"):
        return
    import concourse.hw_specs as hw_specs
    orig = hw_specs.get_activation_tables

    def patched(arch):
        tables = dict(orig(arch))
        ln = mybir.ActivationFunctionType.Ln
        ex = mybir.ActivationFunctionType.Exp
        for name, fns in tables.items():
            if name != "natural_log_exp_and_others":
                tables[name] = fns - {ln, ex}
        return tables

    hw_specs.get_activation_tables = patched
    bacc.get_activation_tables = patched
    _CACHED["act_patched"] = True

B, N, H, W = 4, 256, 128, 128
PIX = H * W // 2          # pixels per core (half plane) = 8192
F = 512                   # pixels per tile
NT = PIX // F             # 16 tiles
BIG = 1024.0
THRESH = -1014.5          # q <= 9 - BIG

f32 = mybir.dt.float32
f32r = mybir.dt.float32r
bf16 = mybir.dt.bfloat16

_CACHED = {}


def _build_program():
    _patch_act_tables()
    nc = bacc.Bacc("TRN2", target_bir_lowering=False, debug=False, num_devices=8)

    a_d = nc.dram_tensor("alpha", [N, PIX], f32, kind="ExternalInput")
    c_d = nc.dram_tensor("color", [N, 3, PIX], f32, kind="ExternalInput")
    p_d = nc.dram_tensor("par", [N, 8], f32, kind="ExternalInput")
    u2b_d = nc.dram_tensor("U2B", [128, 128], bf16, kind="ExternalInput")
    onb_d = nc.dram_tensor("ONESB", [128, 128], bf16, kind="ExternalInput")
    thr_d = nc.dram_tensor("THR", [128, 1], f32, kind="ExternalInput")
    p64_d = nc.dram_tensor("P64", [1, 128], bf16, kind="ExternalInput")
    neg01_d = nc.dram_tensor("NEG01", [128, 1], f32, kind="ExternalInput")
    usr_d = nc.dram_tensor("USR", [128, 128], f32r, kind="ExternalInput")
    onr_d = nc.dram_tensor("ONESR", [128, 128], f32r, kind="ExternalInput")
    stc_d = nc.dram_tensor("STC", [128, 4, 8], f32r, kind="ExternalInput")
    bgw_d = nc.dram_tensor("BGW", [1, 8], f32r, kind="ExternalInput")
    z8_d = nc.dram_tensor("Z8", [128, 8], f32r, kind="ExternalInput")
    out_d = nc.dram_tensor("out", [4, PIX], f32, kind="ExternalOutput")

    Ln = mybir.ActivationFunctionType.Ln
    Exp = mybir.ActivationFunctionType.Exp
    Sign = mybir.ActivationFunctionType.Sign
    MUL = mybir.AluOpType.mult
    LE = mybir.AluOpType.is_le
    GT = mybir.AluOpType.is_gt

    with tile.TileContext(nc) as tc:
        with (
            tc.tile_pool(name="cst", bufs=1) as cst,
            tc.tile_pool(name="io", bufs=5) as io,
            tc.tile_pool(name="wk", bufs=3) as wk,
            tc.tile_pool(name="psq", bufs=2, space="PSUM") as psq,
            tc.tile_pool(name="psl", bufs=1, space="PSUM") as psl,
            tc.tile_pool(name="psr", bufs=2, space="PSUM") as psr,
        ):
            # ---- constants / per-core setup ----
            hp = tc.high_priority(offset=100000)
            hp.__enter__()
            u2b = cst.tile([128, 128], bf16)
            nc.sync.dma_start(u2b[:], u2b_d[:])
            onb = cst.tile([128, 128], bf16)
            nc.sync.dma_start(onb[:], onb_d[:])
            usr = cst.tile([128, 128], f32r)
            nc.sync.dma_start(usr[:], usr_d[:])
            onr = cst.tile([128, 128], f32r)
            nc.sync.dma_start(onr[:], onr_d[:])
            stc = cst.tile([128, 4, 8], f32r)
            nc.sync.dma_start(stc[:], stc_d[:])
            bgw = cst.tile([1, 8], f32r)
            nc.sync.dma_start(bgw[:], bgw_d[:])
            thr = cst.tile([128, 1], f32)
            nc.sync.dma_start(thr[:], thr_d[:])
            p64 = cst.tile([1, 128], bf16)
            nc.sync.dma_start(p64[:], p64_d[:])
            onerow = cst.tile([1, F], bf16)
            nc.vector.memset(onerow[:], 1.0)
            neg01 = cst.tile([128, 1], f32)
            nc.sync.dma_start(neg01[:], neg01_d[:])

            par0 = cst.tile([128, 8], f32)
            nc.sync.dma_start(par0[:], p_d[0:128, :])
            par1 = cst.tile([128, 8], f32)
            nc.sync.dma_start(par1[:], p_d[128:256, :])
            st3l = cst.tile([128, 8], f32r)
            nc.sync.dma_start(st3l[:], z8_d[:])
            st3h = cst.tile([128, 8], f32r)
            nc.sync.dma_start(st3h[:], z8_d[:])
            nc.vector.tensor_tensor(st3l[:, 4:5], par0[:, 2:3], par0[:, 3:4], MUL)
            nc.vector.tensor_tensor(st3h[:, 4:5], par1[:, 2:3], par1[:, 3:4], MUL)
            hp.__exit__(None, None, None)

            # ---- main loop over pixel tiles ----
            for t in range(NT):
                px = bass.ts(t, F)

                At = io.tile([128, 2, F], f32, tag="A")
                Ct = io.tile([128, 2, 3, F], f32, tag="C")
                with tc.high_priority(offset=60):
                    nc.sync.dma_start(
                        At[:],
                        bass.AP(a_d, t * F, [[PIX, 128], [128 * PIX, 2], [1, F]]),
                    )
                    for h in range(2):
                        nc.sync.dma_start(
                            Ct[:, h],
                            bass.AP(
                                c_d, h * 128 * 3 * PIX + t * F,
                                [[3 * PIX, 128], [PIX, 3], [1, F]],
                            ),
                        )

                # msign = sign(A - 0.1) in {-1,+1}; mask algebra folded into
                # halved stationaries, per-row thresholds and a +64 rank-1.
                ms = wk.tile([128, 2, F], bf16, tag="ms")
                nc.scalar.activation(ms[:], At[:], Sign, bias=neg01[:])

                q2 = psq.tile([128, 2, F], f32, tag="q2")
                nc.tensor.matmul(q2[:, 1, :], u2b[:], ms[:, 1, :], start=True, stop=True)
                nc.tensor.matmul(q2[:, 0, :], u2b[:], ms[:, 0, :], start=True, stop=False)
                nc.tensor.matmul(q2[:, 0, :], onb[:], ms[:, 1, :], start=False, stop=False)
                nc.tensor.matmul(q2[:, 0, :], p64[:], onerow[:], start=False, stop=True)

                # ae = (q <= thr) * A over both halves at once
                aet = wk.tile([128, 2, F], f32, tag="aet")
                nc.vector.scalar_tensor_tensor(aet[:], q2[:], thr[:], At[:], LE, MUL)

                # lg = ln(1 - ae)
                lgt = wk.tile([128, 2, F], f32r, tag="lgt")
                nc.scalar.activation(lgt[:], aet[:], Ln, bias=1.0, scale=-1.0)

                # suffix log-sums
                Lx = psl.tile([128, 2, F], f32, tag="Lx")
                nc.tensor.matmul(Lx[:, 1, :], usr[:], lgt[:, 1, :], start=True, stop=True)
                nc.tensor.matmul(Lx[:, 0, :], usr[:], lgt[:, 0, :], start=True, stop=False)
                nc.tensor.matmul(Lx[:, 0, :], onr[:], lgt[:, 1, :], start=False, stop=True)

                Txt = wk.tile([128, 2, F], f32, tag="Txt")
                nc.scalar.activation(Txt[:], Lx[:], Exp)

                # w = ae * Texcl
                wt = wk.tile([128, 2, 1, F], f32r, tag="wt")
                nc.vector.tensor_tensor(wt[:, :, 0, :], aet[:], Txt[:], MUL)

                # Z = w (broadcast over channel) * color, one op
                Zt = wk.tile([128, 2, 3, F], f32r, tag="Zt")
                nc.vector.tensor_tensor(
                    Zt[:], wt[:].to_broadcast([128, 2, 3, F]), Ct[:], MUL
                )

                # reductions: row0 sum(lg), rows 1-3 canvas rgb, row 4 den
                red = psr.tile([8, F], f32, tag="red")
                nc.tensor.matmul(red[:], stc[:, 0, :], Zt[:, 1, 0, :], start=True, stop=False)
                nc.tensor.matmul(red[:], stc[:, 0, :], Zt[:, 0, 0, :], start=False, stop=False)
                nc.tensor.matmul(red[:], stc[:, 1, :], Zt[:, 1, 1, :], start=False, stop=False)
                nc.tensor.matmul(red[:], stc[:, 1, :], Zt[:, 0, 1, :], start=False, stop=False)
                nc.tensor.matmul(red[:], stc[:, 2, :], Zt[:, 1, 2, :], start=False, stop=False)
                nc.tensor.matmul(red[:], stc[:, 2, :], Zt[:, 0, 2, :], start=False, stop=False)
                nc.tensor.matmul(red[:], st3h[:], wt[:, 1, 0, :], start=False, stop=False)
                nc.tensor.matmul(red[:], st3l[:], wt[:, 0, 0, :], start=False, stop=False)
                nc.tensor.matmul(red[:], stc[:, 3, :], lgt[:, 1, :], start=False, stop=False)
                nc.tensor.matmul(red[:], stc[:, 3, :], lgt[:, 0, :], start=False, stop=False)

                # background transmittance onto rows 1-4 via rank-1 matmul
                bg = wk.tile([1, F], f32r, tag="bg")
                nc.scalar.activation(bg[:], red[0:1, :], Exp)
                nc.tensor.matmul(red[:], bgw[:], bg[:], start=False, stop=True)

                outt = wk.tile([8, F], f32, tag="outt")
                nc.scalar.copy(outt[:], red[:])
                nc.sync.dma_start(out_d[:, px], outt[1:5, :])

    nc.compile()
    return nc


def _get_program():
    if "nc" not in _CACHED:
        _CACHED["nc"] = _build_program()
    return _CACHED["nc"]


def _consts():
    if "consts" in _CACHED:
        return _CACHED["consts"]
    tri = np.tril(np.ones((128, 128), np.float32), -1)
    # halved: the matmul consumes msign in {-1,+1}; q = U2h@msign + r
    u2 = 0.5 * tri - (BIG / 2) * np.eye(128, dtype=np.float32)
    ones = np.ones((128, 128), np.float32)
    r = u2.sum(axis=0).astype(np.float32).reshape(128, 1)   # per-row offset
    thr = np.float32(THRESH) - r
    stc = np.zeros((128, 4, 8), np.float32)
    stc[:, 0, 1] = 1.0   # canvas r -> row 1
    stc[:, 1, 2] = 1.0   # canvas g -> row 2
    stc[:, 2, 3] = 1.0   # canvas b -> row 3
    stc[:, 3, 0] = 1.0   # sum(lg) -> row 0
    bgw = np.zeros((1, 8), np.float32)
    bgw[0, 1:5] = 1.0
    consts = {
        "U2B": u2.astype(ml_dtypes.bfloat16),
        "ONESB": (0.5 * ones).astype(ml_dtypes.bfloat16),
        "THR": thr,
        "P64": np.full((1, 128), 64.0, ml_dtypes.bfloat16),
        "NEG01": np.full((128, 1), -np.float32(0.1), np.float32),
        "USR": tri,
        "ONESR": ones,
        "STC": stc,
        "BGW": bgw,
        "Z8": np.zeros((128, 8), np.float32),
    }
    _CACHED["consts"] = consts
    return consts


def _make_in_maps(color_stroke, alpha, params):
    consts = _consts()
    in_maps = []
    for core in range(8):
        b, half = core // 2, core % 2
        r0 = half * (H // 2)
        a = np.ascontiguousarray(
            alpha[b, :, 0, r0 : r0 + H // 2, :].reshape(N, PIX)
        )
        c = np.ascontiguousarray(
            color_stroke[b, :, :, r0 : r0 + H // 2, :].reshape(N, 3, PIX)
        )
        p = np.ascontiguousarray(params[b])
        in_maps.append({"alpha": a, "color": c, "par": p, **consts})
    return in_maps


def kernel(color_stroke, alpha, params, _trace=False, _trace_kwargs=None):
    color_stroke = np.asarray(color_stroke, dtype=np.float32)
    alpha = np.asarray(alpha, dtype=np.float32)
    params = np.asarray(params, dtype=np.float32)

    nc = _get_program()
    in_maps = _make_in_maps(color_stroke, alpha, params)
    res = run_bass_kernel_spmd(
        nc, in_maps, list(range(8)), trace=_trace, **(_trace_kwargs or {})
    )
    _CACHED["last_result"] = res

    canvas = np.empty((B, 3, H, W), np.float32)
    den = np.empty((B, 1, H, W), np.float32)
    for core in range(8):
        b, half = core // 2, core % 2
        r0 = half * (H // 2)
        o = res.results[core]["out"]
        canvas[b, :, r0 : r0 + H // 2, :] = o[0:3].reshape(3, H // 2, W)
        den[b, 0, r0 : r0 + H // 2, :] = o[3].reshape(H // 2, W)
    return canvas, den


# revision 16
# speedup vs baseline: 1.0373x; 1.0213x over previous
"""Trainium2 Bass kernel for AttnPainterOilDensity (per-pixel top-10 stroke
selection + back-to-front alpha compositing).

Math (per pixel, strokes n = 0..255):
  m_n   = alpha_n > 0.1
  E_n   = #{k > n : m_k}                      (visible strokes in front)
  sel_n = m_n and E_n <= 9                    (the last 10 visible strokes)
  ae_n  = alpha_n * sel_n
  lg_n  = ln(1 - ae_n)                        (0 for unselected)
  Lx_n  = sum_{k>n} lg_k ;  Texcl_n = exp(Lx_n)
  w_n   = ae_n * Texcl_n                      (compositing weight)
  canvas_c = sum_n w_n * color_{n,c} + exp(sum_n lg_n)
  den      = sum_n w_n * s_n        + exp(sum_n lg_n),  s_n = p2_n * p3_n

Layout: stroke-major tiles [128 strokes, 512 pixels]; the per-pixel suffix
counts/sums over the stroke (partition) axis run on the PE via triangular
constant stationaries; selection via one fused scalar_tensor_tensor; ln/exp
on ACT; per-stroke reductions back to per-pixel rows via small matmuls.

Sharding: 8 cores = (batch b = core//2) x (half of the 128x128 plane).
"""

import numpy as np
import ml_dtypes

import concourse.bacc as bacc
import concourse.bass as bass
import concourse.tile as tile
from concourse import mybir
from concourse.bass_utils import run_bass_kernel_spmd


def _patch_act_tables():
    # Force Ln and Exp onto the shared natural_log_exp_and_others set so the
    # per-tile Ln -> Exp alternation doesn't reload ACT tables 2x per tile.
    if _CACHED.get("act_patched"):
        return
    import concourse.hw_specs as hw_specs
    orig = hw_specs.get_activation_tables

    def patched(arch):
        tables = dict(orig(arch))
        ln = mybir.ActivationFunctionType.Ln
        ex = mybir.ActivationFunctionType.Exp
        for name, fns in tables.items():
            if name != "natural_log_exp_and_others":
                tables[name] = fns - {ln, ex}
        return tables

    hw_specs.get_activation_tables = patched
    bacc.get_activation_tables = patched
    _CACHED["act_patched"] = True

B, N, H, W = 4, 256, 128, 128
PIX = H * W // 2          # pixels per core (half plane) = 8192
F = 512                   # pixels per tile
NT = PIX // F             # 16 tiles
BIG = 1024.0
THRESH = -1014.5          # q <= 9 - BIG

f32 = mybir.dt.float32
f32r = mybir.dt.float32r
bf16 = mybir.dt.bfloat16

_CACHED = {}


def _build_program():
    _patch_act_tables()
    nc = bacc.Bacc("TRN2", target_bir_lowering=False, debug=False, num_devices=8)

    a_d = nc.dram_tensor("alpha", [N, PIX], f32, kind="ExternalInput")
    c_d = nc.dram_tensor("color", [N, 3, PIX], f32, kind="ExternalInput")
    p_d = nc.dram_tensor("par", [N, 8], f32, kind="ExternalInput")
    u2b_d = nc.dram_tensor("U2B", [128, 128], bf16, kind="ExternalInput")
    onb_d = nc.dram_tensor("ONESB", [128, 128], bf16, kind="ExternalInput")
    thr_d = nc.dram_tensor("THR", [128, 1], f32, kind="ExternalInput")
    p64_d = nc.dram_tensor("P64", [1, 128], bf16, kind="ExternalInput")
    neg01_d = nc.dram_tensor("NEG01", [128, 1], f32, kind="ExternalInput")
    usr_d = nc.dram_tensor("USR", [128, 128], f32r, kind="ExternalInput")
    onr_d = nc.dram_tensor("ONESR", [128, 128], f32r, kind="ExternalInput")
    stc_d = nc.dram_tensor("STC", [128, 4, 8], f32r, kind="ExternalInput")
    bgw_d = nc.dram_tensor("BGW", [1, 8], f32r, kind="ExternalInput")
    z8_d = nc.dram_tensor("Z8", [128, 8], f32r, kind="ExternalInput")
    out_d = nc.dram_tensor("out", [4, PIX], f32, kind="ExternalOutput")

    Ln = mybir.ActivationFunctionType.Ln
    Exp = mybir.ActivationFunctionType.Exp
    Sign = mybir.ActivationFunctionType.Sign
    MUL = mybir.AluOpType.mult
    LE = mybir.AluOpType.is_le
    GT = mybir.AluOpType.is_gt

    with tile.TileContext(nc) as tc:
        with (
            tc.tile_pool(name="cst", bufs=1) as cst,
            tc.tile_pool(name="io", bufs=5) as io,
            tc.tile_pool(name="wk", bufs=3) as wk,
            tc.tile_pool(name="psq", bufs=3, space="PSUM") as psq,
            tc.tile_pool(name="psl", bufs=3, space="PSUM") as psl,
            tc.tile_pool(name="psr", bufs=2, space="PSUM") as psr,
        ):
            # ---- constants / per-core setup ----
            hp = tc.high_priority(offset=100000)
            hp.__enter__()
            u2b = cst.tile([128, 128], bf16)
            nc.sync.dma_start(u2b[:], u2b_d[:])
            onb = cst.tile([128, 128], bf16)
            nc.sync.dma_start(onb[:], onb_d[:])
            usr = cst.tile([128, 128], f32r)
            nc.sync.dma_start(usr[:], usr_d[:])
            onr = cst.tile([128, 128], f32r)
            nc.sync.dma_start(onr[:], onr_d[:])
            stc = cst.tile([128, 4, 8], f32r)
            nc.sync.dma_start(stc[:], stc_d[:])
            bgw = cst.tile([1, 8], f32r)
            nc.sync.dma_start(bgw[:], bgw_d[:])
            thr = cst.tile([128, 1], f32)
            nc.sync.dma_start(thr[:], thr_d[:])
            p64 = cst.tile([1, 128], bf16)
            nc.sync.dma_start(p64[:], p64_d[:])
            onerow = cst.tile([1, F], bf16)
            nc.vector.memset(onerow[:], 1.0)
            neg01 = cst.tile([128, 1], f32)
            nc.sync.dma_start(neg01[:], neg01_d[:])

            par0 = cst.tile([128, 8], f32)
            nc.sync.dma_start(par0[:], p_d[0:128, :])
            par1 = cst.tile([128, 8], f32)
            nc.sync.dma_start(par1[:], p_d[128:256, :])
            st3l = cst.tile([128, 8], f32r)
            nc.sync.dma_start(st3l[:], z8_d[:])
            st3h = cst.tile([128, 8], f32r)
            nc.sync.dma_start(st3h[:], z8_d[:])
            nc.vector.tensor_tensor(st3l[:, 4:5], par0[:, 2:3], par0[:, 3:4], MUL)
            nc.vector.tensor_tensor(st3h[:, 4:5], par1[:, 2:3], par1[:, 3:4], MUL)
            hp.__exit__(None, None, None)

            # ---- main loop over pixel tiles ----
            FC = F // 2  # psum chunk width
            for t in range(NT):
                px = bass.ts(t, F)

                At = io.tile([128, 2, F], f32, tag="A")
                Ct = io.tile([128, 2, 3, F], f32, tag="C")
                with tc.high_priority(offset=60):
                    nc.sync.dma_start(
                        At[:],
                        bass.AP(a_d, t * F, [[PIX, 128], [128 * PIX, 2], [1, F]]),
                    )
                    for h in range(2):
                        nc.sync.dma_start(
                            Ct[:, h],
                            bass.AP(
                                c_d, h * 128 * 3 * PIX + t * F,
                                [[3 * PIX, 128], [PIX, 3], [1, F]],
                            ),
                        )

                # msign = sign(A - 0.1) in {-1,+1}; mask algebra folded into
                # halved stationaries, per-row thresholds and +64 rank-1s.
                ms = wk.tile([128, 2, F], bf16, tag="ms")
                nc.scalar.activation(ms[:], At[:], Sign, bias=neg01[:])

                aet = wk.tile([128, 2, F], f32, tag="aet")
                lgt = wk.tile([128, 2, F], f32r, tag="lgt")
                Txt = wk.tile([128, 2, F], f32, tag="Txt")
                wt = wk.tile([128, 2, 1, F], f32r, tag="wt")
                Zt = wk.tile([128, 2, 3, F], f32r, tag="Zt")
                red = psr.tile([8, F], f32, tag="red")

                for c in range(2):
                    cs = bass.ts(c, FC)

                    q2 = psq.tile([128, 2, FC], f32, tag="qc")
                    nc.tensor.matmul(q2[:, 1, :], u2b[:], ms[:, 1, cs], start=True, stop=True)
                    nc.tensor.matmul(q2[:, 0, :], u2b[:], ms[:, 0, cs], start=True, stop=False)
                    nc.tensor.matmul(q2[:, 0, :], onb[:], ms[:, 1, cs], start=False, stop=False)
                    nc.tensor.matmul(q2[:, 0, :], p64[:], onerow[0:1, cs], start=False, stop=True)

                    # ae = (q <= thr) * A
                    nc.vector.scalar_tensor_tensor(
                        aet[:, :, cs], q2[:], thr[:], At[:, :, cs], LE, MUL
                    )
                    # lg = ln(1 - ae)
                    nc.scalar.activation(
                        lgt[:, :, cs], aet[:, :, cs], Ln, bias=1.0, scale=-1.0
                    )
                    # suffix log-sums
                    Lx = psl.tile([128, 2, FC], f32, tag="lxc")
                    nc.tensor.matmul(Lx[:, 1, :], usr[:], lgt[:, 1, cs], start=True, stop=True)
                    nc.tensor.matmul(Lx[:, 0, :], usr[:], lgt[:, 0, cs], start=True, stop=False)
                    nc.tensor.matmul(Lx[:, 0, :], onr[:], lgt[:, 1, cs], start=False, stop=True)

                    nc.scalar.activation(Txt[:, :, cs], Lx[:], Exp)

                    # w = ae * Texcl
                    nc.vector.tensor_tensor(
                        wt[:, :, 0, cs], aet[:, :, cs], Txt[:, :, cs], MUL
                    )
                    # Z = w (broadcast over channels) * color
                    nc.vector.tensor_tensor(
                        Zt[:, :, :, cs],
                        wt[:, :, :, cs].to_broadcast([128, 2, 3, FC]),
                        Ct[:, :, :, cs],
                        MUL,
                    )
                    # reductions: row0 sum(lg), rows 1-3 canvas rgb, row 4 den
                    nc.tensor.matmul(red[:, cs], stc[:, 0, :], Zt[:, 1, 0, cs], start=(c == 0), stop=False, skip_group_check=True)
                    nc.tensor.matmul(red[:, cs], stc[:, 0, :], Zt[:, 0, 0, cs], start=False, stop=False, skip_group_check=True)
                    nc.tensor.matmul(red[:, cs], stc[:, 1, :], Zt[:, 1, 1, cs], start=False, stop=False, skip_group_check=True)
                    nc.tensor.matmul(red[:, cs], stc[:, 1, :], Zt[:, 0, 1, cs], start=False, stop=False, skip_group_check=True)
                    nc.tensor.matmul(red[:, cs], stc[:, 2, :], Zt[:, 1, 2, cs], start=False, stop=False, skip_group_check=True)
                    nc.tensor.matmul(red[:, cs], stc[:, 2, :], Zt[:, 0, 2, cs], start=False, stop=False, skip_group_check=True)
                    nc.tensor.matmul(red[:, cs], st3h[:], wt[:, 1, 0, cs], start=False, stop=False, skip_group_check=True)
                    nc.tensor.matmul(red[:, cs], st3l[:], wt[:, 0, 0, cs], start=False, stop=False, skip_group_check=True)
                    nc.tensor.matmul(red[:, cs], stc[:, 3, :], lgt[:, 1, cs], start=False, stop=False, skip_group_check=True)
                    nc.tensor.matmul(red[:, cs], stc[:, 3, :], lgt[:, 0, cs], start=False, stop=False, skip_group_check=True)

                # background transmittance onto rows 1-4 via rank-1 matmul
                bg = wk.tile([1, F], f32r, tag="bg")
                nc.scalar.activation(bg[:], red[0:1, :], Exp)
                nc.tensor.matmul(red[:], bgw[:], bg[:], start=False, stop=True, skip_group_check=True)

                outt = wk.tile([8, F], f32, tag="outt")
                nc.scalar.copy(outt[:], red[:])
                nc.sync.dma_start(out_d[:, px], outt[1:5, :])

    nc.compile()
    return nc


def _get_program():
    if "nc" not in _CACHED:
        _CACHED["nc"] = _build_program()
    return _CACHED["nc"]


def _consts():
    if "consts" in _CACHED:
        return _CACHED["consts"]
    tri = np.tril(np.ones((128, 128), np.float32), -1)
    # halved: the matmul consumes msign in {-1,+1}; q = U2h@msign + r
    u2 = 0.5 * tri - (BIG / 2) * np.eye(128, dtype=np.float32)
    ones = np.ones((128, 128), np.float32)
    r = u2.sum(axis=0).astype(np.float32).reshape(128, 1)   # per-row offset
    thr = np.float32(THRESH) - r
    stc = np.zeros((128, 4, 8), np.float32)
    stc[:, 0, 1] = 1.0   # canvas r -> row 1
    stc[:, 1, 2] = 1.0   # canvas g -> row 2
    stc[:, 2, 3] = 1.0   # canvas b -> row 3
    stc[:, 3, 0] = 1.0   # sum(lg) -> row 0
    bgw = np.zeros((1, 8), np.float32)
    bgw[0, 1:5] = 1.0
    consts = {
        "U2B": u2.astype(ml_dtypes.bfloat16),
        "ONESB": (0.5 * ones).astype(ml_dtypes.bfloat16),
        "THR": thr,
        "P64": np.full((1, 128), 64.0, ml_dtypes.bfloat16),
        "NEG01": np.full((128, 1), -np.float32(0.1), np.float32),
        "USR": tri,
        "ONESR": ones,
        "STC": stc,
        "BGW": bgw,
        "Z8": np.zeros((128, 8), np.float32),
    }
    _CACHED["consts"] = consts
    return consts


def _make_in_maps(color_stroke, alpha, params):
    consts = _consts()
    in_maps = []
    for core in range(8):
        b, half = core // 2, core % 2
        r0 = half * (H // 2)
        a = np.ascontiguousarray(
            alpha[b, :, 0, r0 : r0 + H // 2, :].reshape(N, PIX)
        )
        c = np.ascontiguousarray(
            color_stroke[b, :, :, r0 : r0 + H // 2, :].reshape(N, 3, PIX)
        )
        p = np.ascontiguousarray(params[b])
        in_maps.append({"alpha": a, "color": c, "par": p, **consts})
    return in_maps


def kernel(color_stroke, alpha, params, _trace=False, _trace_kwargs=None):
    color_stroke = np.asarray(color_stroke, dtype=np.float32)
    alpha = np.asarray(alpha, dtype=np.float32)
    params = np.asarray(params, dtype=np.float32)

    nc = _get_program()
    in_maps = _make_in_maps(color_stroke, alpha, params)
    res = run_bass_kernel_spmd(
        nc, in_maps, list(range(8)), trace=_trace, **(_trace_kwargs or {})
    )
    _CACHED["last_result"] = res

    canvas = np.empty((B, 3, H, W), np.float32)
    den = np.empty((B, 1, H, W), np.float32)
    for core in range(8):
        b, half = core // 2, core % 2
        r0 = half * (H // 2)
        o = res.results[core]["out"]
        canvas[b, :, r0 : r0 + H // 2, :] = o[0:3].reshape(3, H // 2, W)
        den[b, 0, r0 : r0 + H // 2, :] = o[3].reshape(H // 2, W)
    return canvas, den


# revision 17
# speedup vs baseline: 1.1118x; 1.0717x over previous
"""Trainium2 Bass kernel for AttnPainterOilDensity (per-pixel top-10 stroke
selection + back-to-front alpha compositing).

Math (per pixel, strokes n = 0..255):
  m_n   = alpha_n > 0.1
  E_n   = #{k > n : m_k}                      (visible strokes in front)
  sel_n = m_n and E_n <= 9                    (the last 10 visible strokes)
  ae_n  = alpha_n * sel_n
  lg_n  = ln(1 - ae_n)                        (0 for unselected)
  Lx_n  = sum_{k>n} lg_k ;  Texcl_n = exp(Lx_n)
  w_n   = ae_n * Texcl_n                      (compositing weight)
  canvas_c = sum_n w_n * color_{n,c} + exp(sum_n lg_n)
  den      = sum_n w_n * s_n        + exp(sum_n lg_n),  s_n = p2_n * p3_n

Layout: stroke-major tiles [128 strokes, 512 pixels]; the per-pixel suffix
counts/sums over the stroke (partition) axis run on the PE via triangular
constant stationaries; selection via one fused scalar_tensor_tensor; ln/exp
on ACT; per-stroke reductions back to per-pixel rows via small matmuls.

Sharding: 8 cores = (batch b = core//2) x (half of the 128x128 plane).
"""

import numpy as np
import ml_dtypes

import concourse.bacc as bacc
import concourse.bass as bass
import concourse.tile as tile
from concourse import mybir
from concourse.bass_utils import run_bass_kernel_spmd


def _patch_act_tables():
    # Force Ln and Exp onto the shared natural_log_exp_and_others set so the
    # per-tile Ln -> Exp alternation doesn't reload ACT tables 2x per tile.
    if _CACHED.get("act_patched"):
        return
    import concourse.hw_specs as hw_specs
    orig = hw_specs.get_activation_tables

    def patched(arch):
        tables = dict(orig(arch))
        ln = mybir.ActivationFunctionType.Ln
        ex = mybir.ActivationFunctionType.Exp
        for name, fns in tables.items():
            if name != "natural_log_exp_and_others":
                tables[name] = fns - {ln, ex}
        return tables

    hw_specs.get_activation_tables = patched
    bacc.get_activation_tables = patched
    _CACHED["act_patched"] = True

B, N, H, W = 4, 256, 128, 128
PIX = H * W // 2          # pixels per core (half plane) = 8192
F = 512                   # pixels per tile
NT = PIX // F             # 16 tiles
BIG = 1024.0
THRESH = -1014.5          # q <= 9 - BIG

f32 = mybir.dt.float32
f32r = mybir.dt.float32r
bf16 = mybir.dt.bfloat16

_CACHED = {}


def _build_program():
    _patch_act_tables()
    nc = bacc.Bacc("TRN2", target_bir_lowering=False, debug=False, num_devices=8)

    a_d = nc.dram_tensor("alpha", [N, PIX], f32, kind="ExternalInput")
    c_d = nc.dram_tensor("color", [N, 3, PIX], f32, kind="ExternalInput")
    p_d = nc.dram_tensor("par", [N, 8], f32, kind="ExternalInput")
    u2b_d = nc.dram_tensor("U2B", [128, 128], bf16, kind="ExternalInput")
    onb_d = nc.dram_tensor("ONESB", [128, 128], bf16, kind="ExternalInput")
    thr_d = nc.dram_tensor("THR", [128, 1], f32, kind="ExternalInput")
    p64_d = nc.dram_tensor("P64", [1, 128], bf16, kind="ExternalInput")
    neg01_d = nc.dram_tensor("NEG01", [128, 1], f32, kind="ExternalInput")
    usr_d = nc.dram_tensor("USR", [128, 128], f32r, kind="ExternalInput")
    onr_d = nc.dram_tensor("ONESR", [128, 128], f32r, kind="ExternalInput")
    stc_d = nc.dram_tensor("STC", [128, 4, 8], f32r, kind="ExternalInput")
    bgw_d = nc.dram_tensor("BGW", [1, 8], f32r, kind="ExternalInput")
    z8_d = nc.dram_tensor("Z8", [128, 8], f32r, kind="ExternalInput")
    out_d = nc.dram_tensor("out", [4, PIX], f32, kind="ExternalOutput")

    Ln = mybir.ActivationFunctionType.Ln
    Exp = mybir.ActivationFunctionType.Exp
    Sign = mybir.ActivationFunctionType.Sign
    MUL = mybir.AluOpType.mult
    LE = mybir.AluOpType.is_le
    GT = mybir.AluOpType.is_gt

    with tile.TileContext(nc) as tc:
        with (
            tc.tile_pool(name="cst", bufs=1) as cst,
            tc.tile_pool(name="io", bufs=5) as io,
            tc.tile_pool(name="wk", bufs=3) as wk,
            tc.tile_pool(name="psq", bufs=3, space="PSUM") as psq,
            tc.tile_pool(name="psl", bufs=3, space="PSUM") as psl,
            tc.tile_pool(name="psr", bufs=2, space="PSUM") as psr,
        ):
            # ---- constants / per-core setup ----
            hp = tc.high_priority(offset=100000)
            hp.__enter__()
            u2b = cst.tile([128, 128], bf16)
            nc.sync.dma_start(u2b[:], u2b_d[:])
            onb = cst.tile([128, 128], bf16)
            nc.sync.dma_start(onb[:], onb_d[:])
            usr = cst.tile([128, 128], f32r)
            nc.sync.dma_start(usr[:], usr_d[:])
            onr = cst.tile([128, 128], f32r)
            nc.sync.dma_start(onr[:], onr_d[:])
            stc = cst.tile([128, 4, 8], f32r)
            nc.sync.dma_start(stc[:], stc_d[:])
            bgw = cst.tile([1, 8], f32r)
            nc.sync.dma_start(bgw[:], bgw_d[:])
            thr = cst.tile([128, 1], f32)
            nc.sync.dma_start(thr[:], thr_d[:])
            p64 = cst.tile([1, 128], bf16)
            nc.sync.dma_start(p64[:], p64_d[:])
            onerow = cst.tile([1, F], bf16)
            nc.vector.memset(onerow[:], 1.0)
            neg01 = cst.tile([128, 1], f32)
            nc.sync.dma_start(neg01[:], neg01_d[:])

            par0 = cst.tile([128, 8], f32)
            nc.sync.dma_start(par0[:], p_d[0:128, :])
            par1 = cst.tile([128, 8], f32)
            nc.sync.dma_start(par1[:], p_d[128:256, :])
            st3l = cst.tile([128, 8], f32r)
            nc.sync.dma_start(st3l[:], z8_d[:])
            st3h = cst.tile([128, 8], f32r)
            nc.sync.dma_start(st3h[:], z8_d[:])
            nc.vector.tensor_tensor(st3l[:, 4:5], par0[:, 2:3], par0[:, 3:4], MUL)
            nc.vector.tensor_tensor(st3h[:, 4:5], par1[:, 2:3], par1[:, 3:4], MUL)
            hp.__exit__(None, None, None)

            # ---- main loop over pixel tiles ----
            FC = F // 2  # psum chunk width
            for t in range(NT):
                px = bass.ts(t, F)

                At = io.tile([128, 2, F], f32, tag="A")
                Ct = io.tile([128, 2, 3, F], f32, tag="C")
                with tc.high_priority(offset=60):
                    nc.sync.dma_start(
                        At[:],
                        bass.AP(a_d, t * F, [[PIX, 128], [128 * PIX, 2], [1, F]]),
                    )
                    for h in range(2):
                        nc.sync.dma_start(
                            Ct[:, h],
                            bass.AP(
                                c_d, h * 128 * 3 * PIX + t * F,
                                [[3 * PIX, 128], [PIX, 3], [1, F]],
                            ),
                        )

                # msign = sign(A - 0.1) in {-1,+1}; mask algebra folded into
                # halved stationaries, per-row thresholds and +64 rank-1s.
                ms = wk.tile([128, 2, F], bf16, tag="ms")
                nc.scalar.activation(ms[:], At[:], Sign, bias=neg01[:])

                aet = wk.tile([128, 2, F], f32, tag="aet")
                lgt = wk.tile([128, 2, F], f32r, tag="lgt")
                Txt = wk.tile([128, 2, F], f32, tag="Txt")
                wt = wk.tile([128, 2, 1, F], f32r, tag="wt")
                Zt = wk.tile([128, 2, 3, F], f32r, tag="Zt")
                red = psr.tile([8, F], f32, tag="red")

                for c in range(2):
                    cs = bass.ts(c, FC)

                    q2 = psq.tile([128, 2, FC], f32, tag="qc")
                    nc.tensor.matmul(q2[:, 1, :], u2b[:], ms[:, 1, cs], start=True, stop=True)
                    nc.tensor.matmul(q2[:, 0, :], u2b[:], ms[:, 0, cs], start=True, stop=False)
                    nc.tensor.matmul(q2[:, 0, :], onb[:], ms[:, 1, cs], start=False, stop=False)
                    nc.tensor.matmul(q2[:, 0, :], p64[:], onerow[0:1, cs], start=False, stop=True)

                    # ae = (q <= thr) * A
                    nc.vector.scalar_tensor_tensor(
                        aet[:, :, cs], q2[:], thr[:], At[:, :, cs], LE, MUL
                    )
                    # lg = ln(1 - ae)
                    nc.scalar.activation(
                        lgt[:, :, cs], aet[:, :, cs], Ln, bias=1.0, scale=-1.0
                    )
                    # suffix log-sums
                    Lx = psl.tile([128, 2, FC], f32, tag="lxc")
                    nc.tensor.matmul(Lx[:, 1, :], usr[:], lgt[:, 1, cs], start=True, stop=True)
                    nc.tensor.matmul(Lx[:, 0, :], usr[:], lgt[:, 0, cs], start=True, stop=False)
                    nc.tensor.matmul(Lx[:, 0, :], onr[:], lgt[:, 1, cs], start=False, stop=True)

                    nc.scalar.activation(Txt[:, :, cs], Lx[:], Exp)

                    # w = ae * Texcl
                    nc.vector.tensor_tensor(
                        wt[:, :, 0, cs], aet[:, :, cs], Txt[:, :, cs], MUL
                    )
                    # Z = w (broadcast over channels) * color
                    nc.vector.tensor_tensor(
                        Zt[:, :, :, cs],
                        wt[:, :, :, cs].to_broadcast([128, 2, 3, FC]),
                        Ct[:, :, :, cs],
                        MUL,
                    )
                    # reductions: row0 sum(lg), rows 1-3 canvas rgb, row 4 den
                    nc.tensor.matmul(red[:, cs], stc[:, 0, :], Zt[:, 1, 0, cs], start=(c == 0), stop=False, skip_group_check=True)
                    nc.tensor.matmul(red[:, cs], stc[:, 0, :], Zt[:, 0, 0, cs], start=False, stop=False, skip_group_check=True)
                    nc.tensor.matmul(red[:, cs], stc[:, 1, :], Zt[:, 1, 1, cs], start=False, stop=False, skip_group_check=True)
                    nc.tensor.matmul(red[:, cs], stc[:, 1, :], Zt[:, 0, 1, cs], start=False, stop=False, skip_group_check=True)
                    nc.tensor.matmul(red[:, cs], stc[:, 2, :], Zt[:, 1, 2, cs], start=False, stop=False, skip_group_check=True)
                    nc.tensor.matmul(red[:, cs], stc[:, 2, :], Zt[:, 0, 2, cs], start=False, stop=False, skip_group_check=True)
                    nc.tensor.matmul(red[:, cs], st3h[:], wt[:, 1, 0, cs], start=False, stop=False, skip_group_check=True)
                    nc.tensor.matmul(red[:, cs], st3l[:], wt[:, 0, 0, cs], start=False, stop=False, skip_group_check=True)
                    nc.tensor.matmul(red[:, cs], stc[:, 3, :], lgt[:, 1, cs], start=False, stop=False, skip_group_check=True)
                    nc.tensor.matmul(red[:, cs], stc[:, 3, :], lgt[:, 0, cs], start=False, stop=False, skip_group_check=True)

                # background transmittance onto rows 1-4 via rank-1 matmul
                bg = wk.tile([1, F], f32r, tag="bg")
                nc.scalar.activation(bg[:], red[0:1, :], Exp)
                nc.tensor.matmul(red[:], bgw[:], bg[:], start=False, stop=True, skip_group_check=True)

                outt = wk.tile([8, F], f32, tag="outt")
                nc.scalar.copy(outt[:], red[:])
                nc.scalar.dma_start(out_d[:, px], outt[1:5, :])

    nc.compile()
    return nc


def _get_program():
    if "nc" not in _CACHED:
        _CACHED["nc"] = _build_program()
    return _CACHED["nc"]


def _consts():
    if "consts" in _CACHED:
        return _CACHED["consts"]
    tri = np.tril(np.ones((128, 128), np.float32), -1)
    # halved: the matmul consumes msign in {-1,+1}; q = U2h@msign + r
    u2 = 0.5 * tri - (BIG / 2) * np.eye(128, dtype=np.float32)
    ones = np.ones((128, 128), np.float32)
    r = u2.sum(axis=0).astype(np.float32).reshape(128, 1)   # per-row offset
    thr = np.float32(THRESH) - r
    stc = np.zeros((128, 4, 8), np.float32)
    stc[:, 0, 1] = 1.0   # canvas r -> row 1
    stc[:, 1, 2] = 1.0   # canvas g -> row 2
    stc[:, 2, 3] = 1.0   # canvas b -> row 3
    stc[:, 3, 0] = 1.0   # sum(lg) -> row 0
    bgw = np.zeros((1, 8), np.float32)
    bgw[0, 1:5] = 1.0
    consts = {
        "U2B": u2.astype(ml_dtypes.bfloat16),
        "ONESB": (0.5 * ones).astype(ml_dtypes.bfloat16),
        "THR": thr,
        "P64": np.full((1, 128), 64.0, ml_dtypes.bfloat16),
        "NEG01": np.full((128, 1), -np.float32(0.1), np.float32),
        "USR": tri,
        "ONESR": ones,
        "STC": stc,
        "BGW": bgw,
        "Z8": np.zeros((128, 8), np.float32),
    }
    _CACHED["consts"] = consts
    return consts


def _make_in_maps(color_stroke, alpha, params):
    consts = _consts()
    in_maps = []
    for core in range(8):
        b, half = core // 2, core % 2
        r0 = half * (H // 2)
        a = np.ascontiguousarray(
            alpha[b, :, 0, r0 : r0 + H // 2, :].reshape(N, PIX)
        )
        c = np.ascontiguousarray(
            color_stroke[b, :, :, r0 : r0 + H // 2, :].reshape(N, 3, PIX)
        )
        p = np.ascontiguousarray(params[b])
        in_maps.append({"alpha": a, "color": c, "par": p, **consts})
    return in_maps


def kernel(color_stroke, alpha, params, _trace=False, _trace_kwargs=None):
    color_stroke = np.asarray(color_stroke, dtype=np.float32)
    alpha = np.asarray(alpha, dtype=np.float32)
    params = np.asarray(params, dtype=np.float32)

    nc = _get_program()
    in_maps = _make_in_maps(color_stroke, alpha, params)
    res = run_bass_kernel_spmd(
        nc, in_maps, list(range(8)), trace=_trace, **(_trace_kwargs or {})
    )
    _CACHED["last_result"] = res

    canvas = np.empty((B, 3, H, W), np.float32)
    den = np.empty((B, 1, H, W), np.float32)
    for core in range(8):
        b, half = core // 2, core % 2
        r0 = half * (H // 2)
        o = res.results[core]["out"]
        canvas[b, :, r0 : r0 + H // 2, :] = o[0:3].reshape(3, H // 2, W)
        den[b, 0, r0 : r0 + H // 2, :] = o[3].reshape(H // 2, W)
    return canvas, den


# revision 18
# speedup vs baseline: 2.0794x; 1.8704x over previous
"""Trainium2 Bass kernel for AttnPainterOilDensity (per-pixel top-10 stroke
selection + back-to-front alpha compositing).

Math (per pixel, strokes n = 0..255):
  m_n   = alpha_n > 0.1
  E_n   = #{k > n : m_k}                      (visible strokes in front)
  sel_n = m_n and E_n <= 9                    (the last 10 visible strokes)
  ae_n  = alpha_n * sel_n
  lg_n  = ln(1 - ae_n)                        (0 for unselected)
  Lx_n  = sum_{k>n} lg_k ;  Texcl_n = exp(Lx_n)
  w_n   = ae_n * Texcl_n                      (compositing weight)
  canvas_c = sum_n w_n * color_{n,c} + exp(sum_n lg_n)
  den      = sum_n w_n * s_n        + exp(sum_n lg_n),  s_n = p2_n * p3_n

Layout: stroke-major tiles [128 strokes, 512 pixels]; the per-pixel suffix
counts/sums over the stroke (partition) axis run on the PE via triangular
constant stationaries; selection via one fused scalar_tensor_tensor; ln/exp
on ACT; per-stroke reductions back to per-pixel rows via small matmuls.

Sharding: 8 cores = (batch b = core//2) x (half of the 128x128 plane).
"""

import numpy as np
import ml_dtypes

import concourse.bacc as bacc
import concourse.bass as bass
import concourse.tile as tile
from concourse import mybir
from concourse.bass_utils import run_bass_kernel_spmd


def _patch_act_tables():
    # Force Ln and Exp onto the shared natural_log_exp_and_others set so the
    # per-tile Ln -> Exp alternation doesn't reload ACT tables 2x per tile.
    if _CACHED.get("act_patched"):
        return
    import concourse.hw_specs as hw_specs
    orig = hw_specs.get_activation_tables

    def patched(arch):
        tables = dict(orig(arch))
        ln = mybir.ActivationFunctionType.Ln
        ex = mybir.ActivationFunctionType.Exp
        for name, fns in tables.items():
            if name != "natural_log_exp_and_others":
                tables[name] = fns - {ln, ex}
        return tables

    hw_specs.get_activation_tables = patched
    bacc.get_activation_tables = patched
    _CACHED["act_patched"] = True

B, N, H, W = 4, 256, 128, 128
PIX = H * W // 2          # pixels per core (half plane) = 8192
F = 512                   # pixels per tile
NT = PIX // F             # 16 tiles
BIG = 1024.0
THRESH = -1014.5          # q <= 9 - BIG

f32 = mybir.dt.float32
f32r = mybir.dt.float32r
bf16 = mybir.dt.bfloat16

_CACHED = {}


def _build_program():
    _patch_act_tables()
    nc = bacc.Bacc("TRN2", target_bir_lowering=False, debug=False, num_devices=8)

    a_d = nc.dram_tensor("alpha", [N, PIX], f32, kind="ExternalInput")
    c_d = nc.dram_tensor("color", [N, 3, PIX], f32, kind="ExternalInput")
    p_d = nc.dram_tensor("par", [N, 8], f32, kind="ExternalInput")
    u2b_d = nc.dram_tensor("U2B", [128, 128], bf16, kind="ExternalInput")
    onb_d = nc.dram_tensor("ONESB", [128, 128], bf16, kind="ExternalInput")
    thr_d = nc.dram_tensor("THR", [128, 1], f32, kind="ExternalInput")
    p64_d = nc.dram_tensor("P64", [1, 128], bf16, kind="ExternalInput")
    neg01_d = nc.dram_tensor("NEG01", [128, 1], f32, kind="ExternalInput")
    usr_d = nc.dram_tensor("USR", [128, 128], f32r, kind="ExternalInput")
    onr_d = nc.dram_tensor("ONESR", [128, 128], f32r, kind="ExternalInput")
    stc_d = nc.dram_tensor("STC", [128, 4, 8], f32r, kind="ExternalInput")
    bgw_d = nc.dram_tensor("BGW", [1, 8], f32r, kind="ExternalInput")
    z8_d = nc.dram_tensor("Z8", [128, 8], f32r, kind="ExternalInput")
    out_d = nc.dram_tensor("out", [4, PIX], f32, kind="ExternalOutput")

    Ln = mybir.ActivationFunctionType.Ln
    Exp = mybir.ActivationFunctionType.Exp
    Sign = mybir.ActivationFunctionType.Sign
    MUL = mybir.AluOpType.mult
    LE = mybir.AluOpType.is_le
    GT = mybir.AluOpType.is_gt

    with tile.TileContext(nc) as tc:
        with (
            tc.tile_pool(name="cst", bufs=1) as cst,
            tc.tile_pool(name="io", bufs=5) as io,
            tc.tile_pool(name="wk", bufs=3) as wk,
            tc.tile_pool(name="psq", bufs=3, space="PSUM") as psq,
            tc.tile_pool(name="psl", bufs=3, space="PSUM") as psl,
            tc.tile_pool(name="psr", bufs=2, space="PSUM") as psr,
        ):
            # ---- constants / per-core setup ----
            hp = tc.high_priority(offset=100000)
            hp.__enter__()
            u2b = cst.tile([128, 128], bf16)
            nc.sync.dma_start(u2b[:], u2b_d[:])
            onb = cst.tile([128, 128], bf16)
            nc.sync.dma_start(onb[:], onb_d[:])
            usr = cst.tile([128, 128], f32r)
            nc.sync.dma_start(usr[:], usr_d[:])
            onr = cst.tile([128, 128], f32r)
            nc.sync.dma_start(onr[:], onr_d[:])
            stc = cst.tile([128, 4, 8], f32r)
            nc.sync.dma_start(stc[:], stc_d[:])
            bgw = cst.tile([1, 8], f32r)
            nc.sync.dma_start(bgw[:], bgw_d[:])
            thr = cst.tile([128, 1], f32)
            nc.sync.dma_start(thr[:], thr_d[:])
            p64 = cst.tile([1, 128], bf16)
            nc.sync.dma_start(p64[:], p64_d[:])
            onerow = cst.tile([1, F], bf16)
            nc.vector.memset(onerow[:], 1.0)
            neg01 = cst.tile([128, 1], f32)
            nc.sync.dma_start(neg01[:], neg01_d[:])

            par0 = cst.tile([128, 8], f32)
            nc.sync.dma_start(par0[:], p_d[0:128, :])
            par1 = cst.tile([128, 8], f32)
            nc.sync.dma_start(par1[:], p_d[128:256, :])
            st3l = cst.tile([128, 8], f32r)
            nc.sync.dma_start(st3l[:], z8_d[:])
            st3h = cst.tile([128, 8], f32r)
            nc.sync.dma_start(st3h[:], z8_d[:])
            nc.vector.tensor_tensor(st3l[:, 4:5], par0[:, 2:3], par0[:, 3:4], MUL)
            nc.vector.tensor_tensor(st3h[:, 4:5], par1[:, 2:3], par1[:, 3:4], MUL)
            hp.__exit__(None, None, None)

            # ---- main loop over pixel tiles ----
            FC = F // 2  # psum chunk width
            for t in range(NT):
                px = bass.ts(t, F)

                At = io.tile([128, 2, F], f32, tag="A")
                Ct = io.tile([128, 2, 3, F], f32, tag="C")
                with tc.high_priority(offset=60):
                    nc.sync.dma_start(
                        At[:],
                        bass.AP(a_d, t * F, [[PIX, 128], [128 * PIX, 2], [1, F]]),
                    )
                    for h in range(2):
                        nc.sync.dma_start(
                            Ct[:, h],
                            bass.AP(
                                c_d, h * 128 * 3 * PIX + t * F,
                                [[3 * PIX, 128], [PIX, 3], [1, F]],
                            ),
                        )

                # msign = sign(A - 0.1) in {-1,+1}; mask algebra folded into
                # halved stationaries, per-row thresholds and +64 rank-1s.
                ms = wk.tile([128, 2, F], bf16, tag="ms")
                nc.scalar.activation(ms[:], At[:], Sign, bias=neg01[:])

                aet = wk.tile([128, 2, F], f32, tag="aet")
                lgt = wk.tile([128, 2, F], f32r, tag="lgt")
                Txt = wk.tile([128, 2, F], f32, tag="Txt")
                wt = wk.tile([128, 2, 1, F], f32r, tag="wt")
                Zt = wk.tile([128, 2, 3, F], f32r, tag="Zt")
                red = psr.tile([8, F], f32, tag="red")

                for c in range(2):
                    cs = bass.ts(c, FC)

                    q2 = psq.tile([128, 2, FC], f32, tag="qc")
                    nc.tensor.matmul(q2[:, 1, :], u2b[:], ms[:, 1, cs], start=True, stop=True)
                    nc.tensor.matmul(q2[:, 0, :], u2b[:], ms[:, 0, cs], start=True, stop=False)
                    nc.tensor.matmul(q2[:, 0, :], onb[:], ms[:, 1, cs], start=False, stop=False)
                    nc.tensor.matmul(q2[:, 0, :], p64[:], onerow[0:1, cs], start=False, stop=True)

                    # ae = (q <= thr) * A
                    nc.vector.scalar_tensor_tensor(
                        aet[:, :, cs], q2[:], thr[:], At[:, :, cs], LE, MUL
                    )
                    # lg = ln(1 - ae)
                    nc.scalar.activation(
                        lgt[:, :, cs], aet[:, :, cs], Ln, bias=1.0, scale=-1.0
                    )
                    # suffix log-sums
                    Lx = psl.tile([128, 2, FC], f32, tag="lxc")
                    nc.tensor.matmul(Lx[:, 1, :], usr[:], lgt[:, 1, cs], start=True, stop=True)
                    nc.tensor.matmul(Lx[:, 0, :], usr[:], lgt[:, 0, cs], start=True, stop=False)
                    nc.tensor.matmul(Lx[:, 0, :], onr[:], lgt[:, 1, cs], start=False, stop=True)

                    nc.scalar.activation(Txt[:, :, cs], Lx[:], Exp)

                    # w = ae * Texcl
                    nc.vector.tensor_tensor(
                        wt[:, :, 0, cs], aet[:, :, cs], Txt[:, :, cs], MUL
                    )
                    # Z = w (broadcast over channels) * color
                    nc.vector.tensor_tensor(
                        Zt[:, :, :, cs],
                        wt[:, :, :, cs].to_broadcast([128, 2, 3, FC]),
                        Ct[:, :, :, cs],
                        MUL,
                    )
                    # reductions: row0 sum(lg), rows 1-3 canvas rgb, row 4 den
                    nc.tensor.matmul(red[:, cs], stc[:, 0, :], Zt[:, 1, 0, cs], start=(c == 0), stop=False, skip_group_check=True)
                    nc.tensor.matmul(red[:, cs], stc[:, 0, :], Zt[:, 0, 0, cs], start=False, stop=False, skip_group_check=True)
                    nc.tensor.matmul(red[:, cs], stc[:, 1, :], Zt[:, 1, 1, cs], start=False, stop=False, skip_group_check=True)
                    nc.tensor.matmul(red[:, cs], stc[:, 1, :], Zt[:, 0, 1, cs], start=False, stop=False, skip_group_check=True)
                    nc.tensor.matmul(red[:, cs], stc[:, 2, :], Zt[:, 1, 2, cs], start=False, stop=False, skip_group_check=True)
                    nc.tensor.matmul(red[:, cs], stc[:, 2, :], Zt[:, 0, 2, cs], start=False, stop=False, skip_group_check=True)
                    nc.tensor.matmul(red[:, cs], st3h[:], wt[:, 1, 0, cs], start=False, stop=False, skip_group_check=True)
                    nc.tensor.matmul(red[:, cs], st3l[:], wt[:, 0, 0, cs], start=False, stop=False, skip_group_check=True)
                    nc.tensor.matmul(red[:, cs], stc[:, 3, :], lgt[:, 1, cs], start=False, stop=False, skip_group_check=True)
                    nc.tensor.matmul(red[:, cs], stc[:, 3, :], lgt[:, 0, cs], start=False, stop=False, skip_group_check=True)

                # background transmittance onto rows 1-4 via rank-1 matmul
                bg = wk.tile([1, F], f32r, tag="bg")
                nc.scalar.activation(bg[:], red[0:1, :], Exp)
                nc.tensor.matmul(red[:], bgw[:], bg[:], start=False, stop=True, skip_group_check=True)

                outt = wk.tile([8, F], f32, tag="outt")
                nc.scalar.copy(outt[:], red[:])
                nc.scalar.dma_start(out_d[:, px], outt[1:5, :])

    nc.compile()
    return nc




# ---------------- fast path: selection confined to the last NS strokes ------
NS = 32          # strokes kept (base NB..256); guarded exactly in kernel()
NB = N - NS      # 224
G = 4            # pixel groups packed along the partition dim
FT = G * F       # pixels per fast tile (2048)
NTF = PIX // FT  # 4 tiles


def _build_fast():
    _patch_act_tables()
    nc = bacc.Bacc("TRN2", target_bir_lowering=False, debug=False, num_devices=8)

    a_d = nc.dram_tensor("alpha32", [NS, PIX], f32, kind="ExternalInput")
    c_d = nc.dram_tensor("color32", [NS, 3, PIX], f32, kind="ExternalInput")
    p_d = nc.dram_tensor("par32", [NS, 8], f32, kind="ExternalInput")
    u2_d = nc.dram_tensor("U2BD", [128, 128], bf16, kind="ExternalInput")
    us_d = nc.dram_tensor("USBD", [128, 128], f32r, kind="ExternalInput")
    thr_d = nc.dram_tensor("THR32", [128, 1], f32, kind="ExternalInput")
    neg01_d = nc.dram_tensor("NEG01", [128, 1], f32, kind="ExternalInput")
    sel_d = nc.dram_tensor("SELS", [128, 4, 20], f32r, kind="ExternalInput")
    sden_d = nc.dram_tensor("SDEN0", [128, 20], f32r, kind="ExternalInput")
    bgw_d = nc.dram_tensor("BGW4", [4, 20], f32r, kind="ExternalInput")
    out_d = nc.dram_tensor("out", [4, PIX], f32, kind="ExternalOutput")

    Ln = mybir.ActivationFunctionType.Ln
    Exp = mybir.ActivationFunctionType.Exp
    Sign = mybir.ActivationFunctionType.Sign
    MUL = mybir.AluOpType.mult
    LE = mybir.AluOpType.is_le

    with tile.TileContext(nc) as tc:
        with (
            tc.tile_pool(name="cst", bufs=1) as cst,
            tc.tile_pool(name="io", bufs=3) as io,
            tc.tile_pool(name="wk", bufs=3) as wk,
            tc.tile_pool(name="psq", bufs=3, space="PSUM") as psq,
            tc.tile_pool(name="psl", bufs=3, space="PSUM") as psl,
            tc.tile_pool(name="psr", bufs=2, space="PSUM") as psr,
        ):
            hp = tc.high_priority(offset=100000)
            hp.__enter__()
            u2b = cst.tile([128, 128], bf16)
            nc.sync.dma_start(u2b[:], u2_d[:])
            usr = cst.tile([128, 128], f32r)
            nc.sync.dma_start(usr[:], us_d[:])
            thr = cst.tile([128, 1], f32)
            nc.sync.dma_start(thr[:], thr_d[:])
            neg01 = cst.tile([128, 1], f32)
            nc.sync.dma_start(neg01[:], neg01_d[:])
            sel = cst.tile([128, 4, 20], f32r)
            nc.sync.dma_start(sel[:], sel_d[:])
            sden = cst.tile([128, 20], f32r)
            nc.sync.dma_start(sden[:], sden_d[:])
            bgw = cst.tile([4, 20], f32r)
            nc.sync.dma_start(bgw[:], bgw_d[:])
            par = cst.tile([NS, 8], f32)
            nc.sync.dma_start(par[:], p_d[:])
            s32 = cst.tile([NS, 1], f32r)
            nc.vector.tensor_tensor(s32[:], par[:, 2:3], par[:, 3:4], MUL)
            for g in range(G):
                nc.vector.tensor_copy(sden[32 * g : 32 * g + 32, 7 + 4 * g : 8 + 4 * g], s32[:])
            hp.__exit__(None, None, None)

            for t in range(NTF):
                base = t * FT
                At = io.tile([128, F], f32, tag="A")
                Ct = io.tile([128, 3, F], f32, tag="C")
                with tc.high_priority(offset=60):
                    for g in range(G):
                        nc.sync.dma_start(
                            At[32 * g : 32 * g + 32, :],
                            bass.AP(a_d, base + g * F, [[PIX, NS], [1, F]]),
                        )
                        nc.sync.dma_start(
                            Ct[32 * g : 32 * g + 32, :, :],
                            bass.AP(c_d, base + g * F, [[3 * PIX, NS], [PIX, 3], [1, F]]),
                        )

                ms = wk.tile([128, F], bf16, tag="ms")
                nc.scalar.activation(ms[:], At[:], Sign, bias=neg01[:])
                q2 = psq.tile([128, F], f32, tag="q")
                nc.tensor.matmul(q2[:], u2b[:], ms[:], start=True, stop=True)
                aet = wk.tile([128, F], f32, tag="aet")
                nc.vector.scalar_tensor_tensor(aet[:], q2[:], thr[:], At[:], LE, MUL)
                lgt = wk.tile([128, F], f32r, tag="lgt")
                nc.scalar.activation(lgt[:], aet[:], Ln, bias=1.0, scale=-1.0)
                Lx = psl.tile([128, F], f32, tag="Lx")
                nc.tensor.matmul(Lx[:], usr[:], lgt[:], start=True, stop=True)
                Txt = wk.tile([128, F], f32, tag="Txt")
                nc.scalar.activation(Txt[:], Lx[:], Exp)
                wt = wk.tile([128, 1, F], f32r, tag="wt")
                nc.vector.tensor_tensor(wt[:, 0, :], aet[:], Txt[:], MUL)
                Zt = wk.tile([128, 3, F], f32r, tag="Zt")
                nc.vector.tensor_tensor(
                    Zt[:], wt[:].to_broadcast([128, 3, F]), Ct[:], MUL
                )

                red = psr.tile([20, F], f32, tag="red")
                nc.tensor.matmul(red[:], sel[:, 0, :], lgt[:], start=True, stop=False)
                nc.tensor.matmul(red[:], sel[:, 1, :], Zt[:, 0, :], start=False, stop=False)
                nc.tensor.matmul(red[:], sel[:, 2, :], Zt[:, 1, :], start=False, stop=False)
                nc.tensor.matmul(red[:], sel[:, 3, :], Zt[:, 2, :], start=False, stop=False)
                nc.tensor.matmul(red[:], sden[:], wt[:, 0, :], start=False, stop=False)
                bg = wk.tile([4, F], f32r, tag="bg")
                nc.scalar.activation(bg[:], red[0:4, :], Exp)
                nc.tensor.matmul(red[:], bgw[:], bg[:], start=False, stop=True)

                outt = wk.tile([20, F], f32, tag="outt")
                nc.scalar.copy(outt[:], red[:])
                for g in range(G):
                    nc.scalar.dma_start(
                        bass.AP(out_d, base + g * F, [[PIX, 4], [1, F]]),
                        outt[4 + 4 * g : 8 + 4 * g, :],
                    )

    nc.compile()
    return nc


def _fast_consts():
    if "fconsts" in _CACHED:
        return _CACHED["fconsts"]
    tri = np.tril(np.ones((NS, NS), np.float32), -1)
    u2h = 0.5 * tri - (BIG / 2) * np.eye(NS, dtype=np.float32)
    u2bd = np.zeros((128, 128), np.float32)
    usbd = np.zeros((128, 128), np.float32)
    for g in range(G):
        sl = slice(32 * g, 32 * g + 32)
        u2bd[sl, sl] = u2h
        usbd[sl, sl] = tri
    r32 = u2h.sum(axis=0)
    thr = np.tile(np.float32(THRESH) - r32, G).reshape(128, 1).astype(np.float32)
    sels = np.zeros((128, 4, 20), np.float32)
    bgw4 = np.zeros((4, 20), np.float32)
    for g in range(G):
        sl = slice(32 * g, 32 * g + 32)
        sels[sl, 0, g] = 1.0          # lgsum -> row g
        sels[sl, 1, 4 + 4 * g] = 1.0  # canvas r
        sels[sl, 2, 5 + 4 * g] = 1.0  # canvas g
        sels[sl, 3, 6 + 4 * g] = 1.0  # canvas b
        bgw4[g, 4 + 4 * g : 8 + 4 * g] = 1.0
    fc = {
        "U2BD": u2bd.astype(ml_dtypes.bfloat16),
        "USBD": usbd,
        "THR32": thr,
        "NEG01": np.full((128, 1), -np.float32(0.1), np.float32),
        "SELS": sels,
        "SDEN0": np.zeros((128, 20), np.float32),
        "BGW4": bgw4,
    }
    _CACHED["fconsts"] = fc
    return fc


def _get_program():
    if "nc" not in _CACHED:
        _CACHED["nc"] = _build_program()
    return _CACHED["nc"]


def _consts():
    if "consts" in _CACHED:
        return _CACHED["consts"]
    tri = np.tril(np.ones((128, 128), np.float32), -1)
    # halved: the matmul consumes msign in {-1,+1}; q = U2h@msign + r
    u2 = 0.5 * tri - (BIG / 2) * np.eye(128, dtype=np.float32)
    ones = np.ones((128, 128), np.float32)
    r = u2.sum(axis=0).astype(np.float32).reshape(128, 1)   # per-row offset
    thr = np.float32(THRESH) - r
    stc = np.zeros((128, 4, 8), np.float32)
    stc[:, 0, 1] = 1.0   # canvas r -> row 1
    stc[:, 1, 2] = 1.0   # canvas g -> row 2
    stc[:, 2, 3] = 1.0   # canvas b -> row 3
    stc[:, 3, 0] = 1.0   # sum(lg) -> row 0
    bgw = np.zeros((1, 8), np.float32)
    bgw[0, 1:5] = 1.0
    consts = {
        "U2B": u2.astype(ml_dtypes.bfloat16),
        "ONESB": (0.5 * ones).astype(ml_dtypes.bfloat16),
        "THR": thr,
        "P64": np.full((1, 128), 64.0, ml_dtypes.bfloat16),
        "NEG01": np.full((128, 1), -np.float32(0.1), np.float32),
        "USR": tri,
        "ONESR": ones,
        "STC": stc,
        "BGW": bgw,
        "Z8": np.zeros((128, 8), np.float32),
    }
    _CACHED["consts"] = consts
    return consts


def _make_in_maps(color_stroke, alpha, params):
    consts = _consts()
    in_maps = []
    for core in range(8):
        b, half = core // 2, core % 2
        r0 = half * (H // 2)
        a = np.ascontiguousarray(
            alpha[b, :, 0, r0 : r0 + H // 2, :].reshape(N, PIX)
        )
        c = np.ascontiguousarray(
            color_stroke[b, :, :, r0 : r0 + H // 2, :].reshape(N, 3, PIX)
        )
        p = np.ascontiguousarray(params[b])
        in_maps.append({"alpha": a, "color": c, "par": p, **consts})
    return in_maps


def kernel(color_stroke, alpha, params, _trace=False, _trace_kwargs=None):
    color_stroke = np.asarray(color_stroke, dtype=np.float32)
    alpha = np.asarray(alpha, dtype=np.float32)
    params = np.asarray(params, dtype=np.float32)

    # Guard: the per-pixel top-10 selection is provably confined to the last
    # NS strokes iff every pixel has >= TOPK visible strokes there.
    a_tail = alpha[:, NB:, 0]                       # [B, NS, H, W]
    cnt_min = (a_tail > np.float32(0.1)).sum(axis=1).min()
    fast = cnt_min >= 10

    if fast:
        if "ncf" not in _CACHED:
            _CACHED["ncf"] = _build_fast()
        nc = _CACHED["ncf"]
        fc = _fast_consts()
        in_maps = []
        for core in range(8):
            b, half = core // 2, core % 2
            r0 = half * (H // 2)
            a = np.ascontiguousarray(
                alpha[b, NB:, 0, r0 : r0 + H // 2, :].reshape(NS, PIX)
            )
            c = np.ascontiguousarray(
                color_stroke[b, NB:, :, r0 : r0 + H // 2, :].reshape(NS, 3, PIX)
            )
            p = np.ascontiguousarray(params[b, NB:])
            in_maps.append({"alpha32": a, "color32": c, "par32": p, **fc})
    else:
        nc = _get_program()
        in_maps = _make_in_maps(color_stroke, alpha, params)

    res = run_bass_kernel_spmd(
        nc, in_maps, list(range(8)), trace=_trace, **(_trace_kwargs or {})
    )
    _CACHED["last_result"] = res

    canvas = np.empty((B, 3, H, W), np.float32)
    den = np.empty((B, 1, H, W), np.float32)
    for core in range(8):
        b, half = core // 2, core % 2
        r0 = half * (H // 2)
        o = res.results[core]["out"]
        canvas[b, :, r0 : r0 + H // 2, :] = o[0:3].reshape(3, H // 2, W)
        den[b, 0, r0 : r0 + H // 2, :] = o[3].reshape(H // 2, W)
    return canvas, den


# revision 20
# speedup vs baseline: 2.7325x; 1.3141x over previous
"""Trainium2 Bass kernel for AttnPainterOilDensity (per-pixel top-10 stroke
selection + back-to-front alpha compositing).

Math (per pixel, strokes n = 0..255):
  m_n   = alpha_n > 0.1
  E_n   = #{k > n : m_k}                      (visible strokes in front)
  sel_n = m_n and E_n <= 9                    (the last 10 visible strokes)
  ae_n  = alpha_n * sel_n
  lg_n  = ln(1 - ae_n)                        (0 for unselected)
  Lx_n  = sum_{k>n} lg_k ;  Texcl_n = exp(Lx_n)
  w_n   = ae_n * Texcl_n                      (compositing weight)
  canvas_c = sum_n w_n * color_{n,c} + exp(sum_n lg_n)
  den      = sum_n w_n * s_n        + exp(sum_n lg_n),  s_n = p2_n * p3_n

Layout: stroke-major tiles [128 strokes, 512 pixels]; the per-pixel suffix
counts/sums over the stroke (partition) axis run on the PE via triangular
constant stationaries; selection via one fused scalar_tensor_tensor; ln/exp
on ACT; per-stroke reductions back to per-pixel rows via small matmuls.

Sharding: 8 cores = (batch b = core//2) x (half of the 128x128 plane).
"""

import numpy as np
import ml_dtypes

import concourse.bacc as bacc
import concourse.bass as bass
import concourse.tile as tile
from concourse import mybir
from concourse.bass_utils import run_bass_kernel_spmd


def _patch_act_tables():
    # Force Ln and Exp onto the shared natural_log_exp_and_others set so the
    # per-tile Ln -> Exp alternation doesn't reload ACT tables 2x per tile.
    if _CACHED.get("act_patched"):
        return
    import concourse.hw_specs as hw_specs
    orig = hw_specs.get_activation_tables

    def patched(arch):
        tables = dict(orig(arch))
        ln = mybir.ActivationFunctionType.Ln
        ex = mybir.ActivationFunctionType.Exp
        for name, fns in tables.items():
            if name != "natural_log_exp_and_others":
                tables[name] = fns - {ln, ex}
        return tables

    hw_specs.get_activation_tables = patched
    bacc.get_activation_tables = patched
    _CACHED["act_patched"] = True

B, N, H, W = 4, 256, 128, 128
PIX = H * W // 2          # pixels per core (half plane) = 8192
F = 512                   # pixels per tile
NT = PIX // F             # 16 tiles
BIG = 1024.0
THRESH = -1014.5          # q <= 9 - BIG

f32 = mybir.dt.float32
f32r = mybir.dt.float32r
bf16 = mybir.dt.bfloat16

_CACHED = {}


def _build_program():
    _patch_act_tables()
    nc = bacc.Bacc("TRN2", target_bir_lowering=False, debug=False, num_devices=8)

    a_d = nc.dram_tensor("alpha", [N, PIX], f32, kind="ExternalInput")
    c_d = nc.dram_tensor("color", [N, 3, PIX], f32, kind="ExternalInput")
    p_d = nc.dram_tensor("par", [N, 8], f32, kind="ExternalInput")
    u2b_d = nc.dram_tensor("U2B", [128, 128], bf16, kind="ExternalInput")
    onb_d = nc.dram_tensor("ONESB", [128, 128], bf16, kind="ExternalInput")
    thr_d = nc.dram_tensor("THR", [128, 1], f32, kind="ExternalInput")
    p64_d = nc.dram_tensor("P64", [1, 128], bf16, kind="ExternalInput")
    neg01_d = nc.dram_tensor("NEG01", [128, 1], f32, kind="ExternalInput")
    usr_d = nc.dram_tensor("USR", [128, 128], f32r, kind="ExternalInput")
    onr_d = nc.dram_tensor("ONESR", [128, 128], f32r, kind="ExternalInput")
    stc_d = nc.dram_tensor("STC", [128, 4, 8], f32r, kind="ExternalInput")
    bgw_d = nc.dram_tensor("BGW", [1, 8], f32r, kind="ExternalInput")
    z8_d = nc.dram_tensor("Z8", [128, 8], f32r, kind="ExternalInput")
    out_d = nc.dram_tensor("out", [4, PIX], f32, kind="ExternalOutput")

    Ln = mybir.ActivationFunctionType.Ln
    Exp = mybir.ActivationFunctionType.Exp
    Sign = mybir.ActivationFunctionType.Sign
    MUL = mybir.AluOpType.mult
    LE = mybir.AluOpType.is_le
    GT = mybir.AluOpType.is_gt

    with tile.TileContext(nc) as tc:
        with (
            tc.tile_pool(name="cst", bufs=1) as cst,
            tc.tile_pool(name="io", bufs=5) as io,
            tc.tile_pool(name="wk", bufs=3) as wk,
            tc.tile_pool(name="psq", bufs=3, space="PSUM") as psq,
            tc.tile_pool(name="psl", bufs=3, space="PSUM") as psl,
            tc.tile_pool(name="psr", bufs=2, space="PSUM") as psr,
        ):
            # ---- constants / per-core setup ----
            hp = tc.high_priority(offset=100000)
            hp.__enter__()
            u2b = cst.tile([128, 128], bf16)
            nc.sync.dma_start(u2b[:], u2b_d[:])
            onb = cst.tile([128, 128], bf16)
            nc.sync.dma_start(onb[:], onb_d[:])
            usr = cst.tile([128, 128], f32r)
            nc.sync.dma_start(usr[:], usr_d[:])
            onr = cst.tile([128, 128], f32r)
            nc.sync.dma_start(onr[:], onr_d[:])
            stc = cst.tile([128, 4, 8], f32r)
            nc.sync.dma_start(stc[:], stc_d[:])
            bgw = cst.tile([1, 8], f32r)
            nc.sync.dma_start(bgw[:], bgw_d[:])
            thr = cst.tile([128, 1], f32)
            nc.sync.dma_start(thr[:], thr_d[:])
            p64 = cst.tile([1, 128], bf16)
            nc.sync.dma_start(p64[:], p64_d[:])
            onerow = cst.tile([1, F], bf16)
            nc.vector.memset(onerow[:], 1.0)
            neg01 = cst.tile([128, 1], f32)
            nc.sync.dma_start(neg01[:], neg01_d[:])

            par0 = cst.tile([128, 8], f32)
            nc.sync.dma_start(par0[:], p_d[0:128, :])
            par1 = cst.tile([128, 8], f32)
            nc.sync.dma_start(par1[:], p_d[128:256, :])
            st3l = cst.tile([128, 8], f32r)
            nc.sync.dma_start(st3l[:], z8_d[:])
            st3h = cst.tile([128, 8], f32r)
            nc.sync.dma_start(st3h[:], z8_d[:])
            nc.vector.tensor_tensor(st3l[:, 4:5], par0[:, 2:3], par0[:, 3:4], MUL)
            nc.vector.tensor_tensor(st3h[:, 4:5], par1[:, 2:3], par1[:, 3:4], MUL)
            hp.__exit__(None, None, None)

            # ---- main loop over pixel tiles ----
            FC = F // 2  # psum chunk width
            for t in range(NT):
                px = bass.ts(t, F)

                At = io.tile([128, 2, F], f32, tag="A")
                Ct = io.tile([128, 2, 3, F], f32, tag="C")
                with tc.high_priority(offset=60):
                    nc.sync.dma_start(
                        At[:],
                        bass.AP(a_d, t * F, [[PIX, 128], [128 * PIX, 2], [1, F]]),
                    )
                    for h in range(2):
                        nc.sync.dma_start(
                            Ct[:, h],
                            bass.AP(
                                c_d, h * 128 * 3 * PIX + t * F,
                                [[3 * PIX, 128], [PIX, 3], [1, F]],
                            ),
                        )

                # msign = sign(A - 0.1) in {-1,+1}; mask algebra folded into
                # halved stationaries, per-row thresholds and +64 rank-1s.
                ms = wk.tile([128, 2, F], bf16, tag="ms")
                nc.scalar.activation(ms[:], At[:], Sign, bias=neg01[:])

                aet = wk.tile([128, 2, F], f32, tag="aet")
                lgt = wk.tile([128, 2, F], f32r, tag="lgt")
                Txt = wk.tile([128, 2, F], f32, tag="Txt")
                wt = wk.tile([128, 2, 1, F], f32r, tag="wt")
                Zt = wk.tile([128, 2, 3, F], f32r, tag="Zt")
                red = psr.tile([8, F], f32, tag="red")

                for c in range(2):
                    cs = bass.ts(c, FC)

                    q2 = psq.tile([128, 2, FC], f32, tag="qc")
                    nc.tensor.matmul(q2[:, 1, :], u2b[:], ms[:, 1, cs], start=True, stop=True)
                    nc.tensor.matmul(q2[:, 0, :], u2b[:], ms[:, 0, cs], start=True, stop=False)
                    nc.tensor.matmul(q2[:, 0, :], onb[:], ms[:, 1, cs], start=False, stop=False)
                    nc.tensor.matmul(q2[:, 0, :], p64[:], onerow[0:1, cs], start=False, stop=True)

                    # ae = (q <= thr) * A
                    nc.vector.scalar_tensor_tensor(
                        aet[:, :, cs], q2[:], thr[:], At[:, :, cs], LE, MUL
                    )
                    # lg = ln(1 - ae)
                    nc.scalar.activation(
                        lgt[:, :, cs], aet[:, :, cs], Ln, bias=1.0, scale=-1.0
                    )
                    # suffix log-sums
                    Lx = psl.tile([128, 2, FC], f32, tag="lxc")
                    nc.tensor.matmul(Lx[:, 1, :], usr[:], lgt[:, 1, cs], start=True, stop=True)
                    nc.tensor.matmul(Lx[:, 0, :], usr[:], lgt[:, 0, cs], start=True, stop=False)
                    nc.tensor.matmul(Lx[:, 0, :], onr[:], lgt[:, 1, cs], start=False, stop=True)

                    nc.scalar.activation(Txt[:, :, cs], Lx[:], Exp)

                    # w = ae * Texcl
                    nc.vector.tensor_tensor(
                        wt[:, :, 0, cs], aet[:, :, cs], Txt[:, :, cs], MUL
                    )
                    # Z = w (broadcast over channels) * color
                    nc.vector.tensor_tensor(
                        Zt[:, :, :, cs],
                        wt[:, :, :, cs].to_broadcast([128, 2, 3, FC]),
                        Ct[:, :, :, cs],
                        MUL,
                    )
                    # reductions: row0 sum(lg), rows 1-3 canvas rgb, row 4 den
                    nc.tensor.matmul(red[:, cs], stc[:, 0, :], Zt[:, 1, 0, cs], start=(c == 0), stop=False, skip_group_check=True)
                    nc.tensor.matmul(red[:, cs], stc[:, 0, :], Zt[:, 0, 0, cs], start=False, stop=False, skip_group_check=True)
                    nc.tensor.matmul(red[:, cs], stc[:, 1, :], Zt[:, 1, 1, cs], start=False, stop=False, skip_group_check=True)
                    nc.tensor.matmul(red[:, cs], stc[:, 1, :], Zt[:, 0, 1, cs], start=False, stop=False, skip_group_check=True)
                    nc.tensor.matmul(red[:, cs], stc[:, 2, :], Zt[:, 1, 2, cs], start=False, stop=False, skip_group_check=True)
                    nc.tensor.matmul(red[:, cs], stc[:, 2, :], Zt[:, 0, 2, cs], start=False, stop=False, skip_group_check=True)
                    nc.tensor.matmul(red[:, cs], st3h[:], wt[:, 1, 0, cs], start=False, stop=False, skip_group_check=True)
                    nc.tensor.matmul(red[:, cs], st3l[:], wt[:, 0, 0, cs], start=False, stop=False, skip_group_check=True)
                    nc.tensor.matmul(red[:, cs], stc[:, 3, :], lgt[:, 1, cs], start=False, stop=False, skip_group_check=True)
                    nc.tensor.matmul(red[:, cs], stc[:, 3, :], lgt[:, 0, cs], start=False, stop=False, skip_group_check=True)

                # background transmittance onto rows 1-4 via rank-1 matmul
                bg = wk.tile([1, F], f32r, tag="bg")
                nc.scalar.activation(bg[:], red[0:1, :], Exp)
                nc.tensor.matmul(red[:], bgw[:], bg[:], start=False, stop=True, skip_group_check=True)

                outt = wk.tile([8, F], f32, tag="outt")
                nc.scalar.copy(outt[:], red[:])
                nc.scalar.dma_start(out_d[:, px], outt[1:5, :])

    nc.compile()
    return nc




# ---------------- fast path: selection confined to the last NS strokes ------
NS = 32          # strokes kept (base NB..256); guarded exactly in kernel()
NB = N - NS      # 224
G = 4            # pixel groups packed along the partition dim
FF = 256         # pixels per group per fast tile
FT = G * FF      # pixels per fast tile (1024)
NTF = PIX // FT  # 8 tiles
PIX4 = PIX // G  # packed free extent (2048)


def _build_fast():
    _patch_act_tables()
    nc = bacc.Bacc("TRN2", target_bir_lowering=False, debug=False, num_devices=8)

    a_d = nc.dram_tensor("alpha32", [128, NTF, FF], f32, kind="ExternalInput")
    c_d = nc.dram_tensor("color32", [128, 3, NTF, FF], f32, kind="ExternalInput")
    p_d = nc.dram_tensor("par32", [NS, 8], f32, kind="ExternalInput")
    u2_d = nc.dram_tensor("U2BD", [128, 128], bf16, kind="ExternalInput")
    us_d = nc.dram_tensor("USBD", [128, 128], f32r, kind="ExternalInput")
    thr_d = nc.dram_tensor("THR32", [128, 1], f32, kind="ExternalInput")
    neg01_d = nc.dram_tensor("NEG01", [128, 1], f32, kind="ExternalInput")
    sel_d = nc.dram_tensor("SELS", [128, 4, 20], f32r, kind="ExternalInput")
    sden_d = nc.dram_tensor("SDEN0", [128, 20], f32r, kind="ExternalInput")
    bgw_d = nc.dram_tensor("BGW4", [4, 20], f32r, kind="ExternalInput")
    out_d = nc.dram_tensor("out", [4, PIX], f32, kind="ExternalOutput")

    Ln = mybir.ActivationFunctionType.Ln
    Exp = mybir.ActivationFunctionType.Exp
    Sign = mybir.ActivationFunctionType.Sign
    MUL = mybir.AluOpType.mult
    LE = mybir.AluOpType.is_le

    with tile.TileContext(nc) as tc:
        with (
            tc.tile_pool(name="cst", bufs=1) as cst,
            tc.tile_pool(name="io", bufs=3) as io,
            tc.tile_pool(name="wk", bufs=3) as wk,
            tc.tile_pool(name="psq", bufs=3, space="PSUM") as psq,
            tc.tile_pool(name="psl", bufs=3, space="PSUM") as psl,
            tc.tile_pool(name="psr", bufs=2, space="PSUM") as psr,
        ):
            hp = tc.high_priority(offset=100000)
            hp.__enter__()
            u2b = cst.tile([128, 128], bf16)
            nc.sync.dma_start(u2b[:], u2_d[:])
            usr = cst.tile([128, 128], f32r)
            nc.sync.dma_start(usr[:], us_d[:])
            thr = cst.tile([128, 1], f32)
            nc.sync.dma_start(thr[:], thr_d[:])
            neg01 = cst.tile([128, 1], f32)
            nc.sync.dma_start(neg01[:], neg01_d[:])
            sel = cst.tile([128, 4, 20], f32r)
            nc.sync.dma_start(sel[:], sel_d[:])
            sden = cst.tile([128, 20], f32r)
            nc.sync.dma_start(sden[:], sden_d[:])
            bgw = cst.tile([4, 20], f32r)
            nc.sync.dma_start(bgw[:], bgw_d[:])
            par = cst.tile([NS, 8], f32)
            nc.sync.dma_start(par[:], p_d[:])
            s32 = cst.tile([NS, 1], f32r)
            nc.vector.tensor_tensor(s32[:], par[:, 2:3], par[:, 3:4], MUL)
            warm = cst.tile([128, 1], f32)
            nc.scalar.activation(warm[:], neg01[:], Sign, bias=0.0)
            nc.scalar.activation(warm[:], neg01[:], Ln, bias=0.0, scale=-10.0)
            nc.scalar.activation(warm[:], warm[:], Exp)
            for g in range(G):
                nc.vector.tensor_copy(sden[32 * g : 32 * g + 32, 7 + 4 * g : 8 + 4 * g], s32[:])
            hp.__exit__(None, None, None)

            for t in range(NTF):
                base = t * FT
                At = io.tile([128, FF], f32, tag="A")
                Ct = io.tile([128, 3, FF], f32, tag="C")
                with tc.high_priority(offset=60):
                    nc.sync.dma_start(At[:], a_d[:, t, :])
                    nc.sync.dma_start(Ct[:], c_d[:, :, t, :])

                ms = wk.tile([128, FF], bf16, tag="ms")
                nc.scalar.activation(ms[:], At[:], Sign, bias=neg01[:])
                q2 = psq.tile([128, FF], f32, tag="q")
                nc.tensor.matmul(q2[:], u2b[:], ms[:], start=True, stop=True)
                aet = wk.tile([128, FF], f32, tag="aet")
                nc.vector.scalar_tensor_tensor(aet[:], q2[:], thr[:], At[:], LE, MUL)
                lgt = wk.tile([128, FF], f32r, tag="lgt")
                nc.scalar.activation(lgt[:], aet[:], Ln, bias=1.0, scale=-1.0)
                Lx = psl.tile([128, FF], f32, tag="Lx")
                nc.tensor.matmul(Lx[:], usr[:], lgt[:], start=True, stop=True)
                Txt = wk.tile([128, FF], f32, tag="Txt")
                nc.scalar.activation(Txt[:], Lx[:], Exp)
                wt = wk.tile([128, 1, FF], f32r, tag="wt")
                nc.vector.tensor_tensor(wt[:, 0, :], aet[:], Txt[:], MUL)
                Zt = wk.tile([128, 3, FF], f32r, tag="Zt")
                nc.vector.tensor_tensor(
                    Zt[:], wt[:].to_broadcast([128, 3, FF]), Ct[:], MUL
                )

                red = psr.tile([20, FF], f32, tag="red")
                nc.tensor.matmul(red[:], sel[:, 0, :], lgt[:], start=True, stop=False)
                nc.tensor.matmul(red[:], sel[:, 1, :], Zt[:, 0, :], start=False, stop=False)
                nc.tensor.matmul(red[:], sel[:, 2, :], Zt[:, 1, :], start=False, stop=False)
                nc.tensor.matmul(red[:], sel[:, 3, :], Zt[:, 2, :], start=False, stop=False)
                nc.tensor.matmul(red[:], sden[:], wt[:, 0, :], start=False, stop=False)
                bg = wk.tile([4, FF], f32r, tag="bg")
                nc.scalar.activation(bg[:], red[0:4, :], Exp)
                nc.tensor.matmul(red[:], bgw[:], bg[:], start=False, stop=True)

                outt = wk.tile([20, FF], f32, tag="outt")
                nc.scalar.copy(outt[:], red[:])
                for g in range(G):
                    nc.scalar.dma_start(
                        bass.AP(out_d, base + g * FF, [[PIX, 4], [1, FF]]),
                        outt[4 + 4 * g : 8 + 4 * g, :],
                    )

    nc.compile()
    return nc


def _fast_consts():
    if "fconsts" in _CACHED:
        return _CACHED["fconsts"]
    tri = np.tril(np.ones((NS, NS), np.float32), -1)
    u2h = 0.5 * tri - (BIG / 2) * np.eye(NS, dtype=np.float32)
    u2bd = np.zeros((128, 128), np.float32)
    usbd = np.zeros((128, 128), np.float32)
    for g in range(G):
        sl = slice(32 * g, 32 * g + 32)
        u2bd[sl, sl] = u2h
        usbd[sl, sl] = tri
    r32 = u2h.sum(axis=0)
    thr = np.tile(np.float32(THRESH) - r32, G).reshape(128, 1).astype(np.float32)
    sels = np.zeros((128, 4, 20), np.float32)
    bgw4 = np.zeros((4, 20), np.float32)
    for g in range(G):
        sl = slice(32 * g, 32 * g + 32)
        sels[sl, 0, g] = 1.0          # lgsum -> row g
        sels[sl, 1, 4 + 4 * g] = 1.0  # canvas r
        sels[sl, 2, 5 + 4 * g] = 1.0  # canvas g
        sels[sl, 3, 6 + 4 * g] = 1.0  # canvas b
        bgw4[g, 4 + 4 * g : 8 + 4 * g] = 1.0
    fc = {
        "U2BD": u2bd.astype(ml_dtypes.bfloat16),
        "USBD": usbd,
        "THR32": thr,
        "NEG01": np.full((128, 1), -np.float32(0.1), np.float32),
        "SELS": sels,
        "SDEN0": np.zeros((128, 20), np.float32),
        "BGW4": bgw4,
    }
    _CACHED["fconsts"] = fc
    return fc


def _get_program():
    if "nc" not in _CACHED:
        _CACHED["nc"] = _build_program()
    return _CACHED["nc"]


def _consts():
    if "consts" in _CACHED:
        return _CACHED["consts"]
    tri = np.tril(np.ones((128, 128), np.float32), -1)
    # halved: the matmul consumes msign in {-1,+1}; q = U2h@msign + r
    u2 = 0.5 * tri - (BIG / 2) * np.eye(128, dtype=np.float32)
    ones = np.ones((128, 128), np.float32)
    r = u2.sum(axis=0).astype(np.float32).reshape(128, 1)   # per-row offset
    thr = np.float32(THRESH) - r
    stc = np.zeros((128, 4, 8), np.float32)
    stc[:, 0, 1] = 1.0   # canvas r -> row 1
    stc[:, 1, 2] = 1.0   # canvas g -> row 2
    stc[:, 2, 3] = 1.0   # canvas b -> row 3
    stc[:, 3, 0] = 1.0   # sum(lg) -> row 0
    bgw = np.zeros((1, 8), np.float32)
    bgw[0, 1:5] = 1.0
    consts = {
        "U2B": u2.astype(ml_dtypes.bfloat16),
        "ONESB": (0.5 * ones).astype(ml_dtypes.bfloat16),
        "THR": thr,
        "P64": np.full((1, 128), 64.0, ml_dtypes.bfloat16),
        "NEG01": np.full((128, 1), -np.float32(0.1), np.float32),
        "USR": tri,
        "ONESR": ones,
        "STC": stc,
        "BGW": bgw,
        "Z8": np.zeros((128, 8), np.float32),
    }
    _CACHED["consts"] = consts
    return consts


def _make_in_maps(color_stroke, alpha, params):
    consts = _consts()
    in_maps = []
    for core in range(8):
        b, half = core // 2, core % 2
        r0 = half * (H // 2)
        a = np.ascontiguousarray(
            alpha[b, :, 0, r0 : r0 + H // 2, :].reshape(N, PIX)
        )
        c = np.ascontiguousarray(
            color_stroke[b, :, :, r0 : r0 + H // 2, :].reshape(N, 3, PIX)
        )
        p = np.ascontiguousarray(params[b])
        in_maps.append({"alpha": a, "color": c, "par": p, **consts})
    return in_maps


def kernel(color_stroke, alpha, params, _trace=False, _trace_kwargs=None):
    color_stroke = np.asarray(color_stroke, dtype=np.float32)
    alpha = np.asarray(alpha, dtype=np.float32)
    params = np.asarray(params, dtype=np.float32)

    # Guard: the per-pixel top-10 selection is provably confined to the last
    # NS strokes iff every pixel has >= TOPK visible strokes there.
    a_tail = alpha[:, NB:, 0]                       # [B, NS, H, W]
    cnt_min = (a_tail > np.float32(0.1)).sum(axis=1).min()
    fast = cnt_min >= 10

    if fast:
        if "ncf" not in _CACHED:
            _CACHED["ncf"] = _build_fast()
        nc = _CACHED["ncf"]
        fc = _fast_consts()
        in_maps = []
        for core in range(8):
            b, half = core // 2, core % 2
            r0 = half * (H // 2)
            a32 = alpha[b, NB:, 0, r0 : r0 + H // 2, :].reshape(NS, PIX)
            a = np.ascontiguousarray(
                a32.reshape(NS, NTF, G, FF).transpose(2, 0, 1, 3).reshape(128, NTF, FF)
            )
            c32 = color_stroke[b, NB:, :, r0 : r0 + H // 2, :].reshape(NS, 3, PIX)
            c = np.ascontiguousarray(
                c32.reshape(NS, 3, NTF, G, FF)
                .transpose(3, 0, 1, 2, 4)
                .reshape(128, 3, NTF, FF)
            )
            p = np.ascontiguousarray(params[b, NB:])
            in_maps.append({"alpha32": a, "color32": c, "par32": p, **fc})
    else:
        nc = _get_program()
        in_maps = _make_in_maps(color_stroke, alpha, params)

    res = run_bass_kernel_spmd(
        nc, in_maps, list(range(8)), trace=_trace, **(_trace_kwargs or {})
    )
    _CACHED["last_result"] = res

    canvas = np.empty((B, 3, H, W), np.float32)
    den = np.empty((B, 1, H, W), np.float32)
    for core in range(8):
        b, half = core // 2, core % 2
        r0 = half * (H // 2)
        o = res.results[core]["out"]
        canvas[b, :, r0 : r0 + H // 2, :] = o[0:3].reshape(3, H // 2, W)
        den[b, 0, r0 : r0 + H // 2, :] = o[3].reshape(H // 2, W)
    return canvas, den


# revision 21
# speedup vs baseline: 3.2478x; 1.1886x over previous
"""Trainium2 Bass kernel for AttnPainterOilDensity (per-pixel top-10 stroke
selection + back-to-front alpha compositing).

Math (per pixel, strokes n = 0..255):
  m_n   = alpha_n > 0.1
  E_n   = #{k > n : m_k}                      (visible strokes in front)
  sel_n = m_n and E_n <= 9                    (the last 10 visible strokes)
  ae_n  = alpha_n * sel_n
  lg_n  = ln(1 - ae_n)                        (0 for unselected)
  Lx_n  = sum_{k>n} lg_k ;  Texcl_n = exp(Lx_n)
  w_n   = ae_n * Texcl_n                      (compositing weight)
  canvas_c = sum_n w_n * color_{n,c} + exp(sum_n lg_n)
  den      = sum_n w_n * s_n        + exp(sum_n lg_n),  s_n = p2_n * p3_n

Layout: stroke-major tiles [128 strokes, 512 pixels]; the per-pixel suffix
counts/sums over the stroke (partition) axis run on the PE via triangular
constant stationaries; selection via one fused scalar_tensor_tensor; ln/exp
on ACT; per-stroke reductions back to per-pixel rows via small matmuls.

Sharding: 8 cores = (batch b = core//2) x (half of the 128x128 plane).
"""

import numpy as np
import ml_dtypes

import concourse.bacc as bacc
import concourse.bass as bass
import concourse.tile as tile
from concourse import mybir
from concourse.bass_utils import run_bass_kernel_spmd


def _patch_act_tables():
    # Force Ln and Exp onto the shared natural_log_exp_and_others set so the
    # per-tile Ln -> Exp alternation doesn't reload ACT tables 2x per tile.
    if _CACHED.get("act_patched"):
        return
    import concourse.hw_specs as hw_specs
    orig = hw_specs.get_activation_tables

    def patched(arch):
        tables = dict(orig(arch))
        ln = mybir.ActivationFunctionType.Ln
        ex = mybir.ActivationFunctionType.Exp
        for name, fns in tables.items():
            if name != "natural_log_exp_and_others":
                tables[name] = fns - {ln, ex}
        return tables

    hw_specs.get_activation_tables = patched
    bacc.get_activation_tables = patched
    _CACHED["act_patched"] = True

B, N, H, W = 4, 256, 128, 128
PIX = H * W // 2          # pixels per core (half plane) = 8192
F = 512                   # pixels per tile
NT = PIX // F             # 16 tiles
BIG = 1024.0
THRESH = -1014.5          # q <= 9 - BIG

f32 = mybir.dt.float32
f32r = mybir.dt.float32r
bf16 = mybir.dt.bfloat16

_CACHED = {}


def _build_program():
    _patch_act_tables()
    nc = bacc.Bacc("TRN2", target_bir_lowering=False, debug=False, num_devices=8)

    a_d = nc.dram_tensor("alpha", [N, PIX], f32, kind="ExternalInput")
    c_d = nc.dram_tensor("color", [N, 3, PIX], f32, kind="ExternalInput")
    p_d = nc.dram_tensor("par", [N, 8], f32, kind="ExternalInput")
    u2b_d = nc.dram_tensor("U2B", [128, 128], bf16, kind="ExternalInput")
    onb_d = nc.dram_tensor("ONESB", [128, 128], bf16, kind="ExternalInput")
    thr_d = nc.dram_tensor("THR", [128, 1], f32, kind="ExternalInput")
    p64_d = nc.dram_tensor("P64", [1, 128], bf16, kind="ExternalInput")
    neg01_d = nc.dram_tensor("NEG01", [128, 1], f32, kind="ExternalInput")
    usr_d = nc.dram_tensor("USR", [128, 128], f32r, kind="ExternalInput")
    onr_d = nc.dram_tensor("ONESR", [128, 128], f32r, kind="ExternalInput")
    stc_d = nc.dram_tensor("STC", [128, 4, 8], f32r, kind="ExternalInput")
    bgw_d = nc.dram_tensor("BGW", [1, 8], f32r, kind="ExternalInput")
    z8_d = nc.dram_tensor("Z8", [128, 8], f32r, kind="ExternalInput")
    out_d = nc.dram_tensor("out", [4, PIX], f32, kind="ExternalOutput")

    Ln = mybir.ActivationFunctionType.Ln
    Exp = mybir.ActivationFunctionType.Exp
    Sign = mybir.ActivationFunctionType.Sign
    MUL = mybir.AluOpType.mult
    LE = mybir.AluOpType.is_le
    GT = mybir.AluOpType.is_gt

    with tile.TileContext(nc) as tc:
        with (
            tc.tile_pool(name="cst", bufs=1) as cst,
            tc.tile_pool(name="io", bufs=5) as io,
            tc.tile_pool(name="wk", bufs=3) as wk,
            tc.tile_pool(name="psq", bufs=3, space="PSUM") as psq,
            tc.tile_pool(name="psl", bufs=3, space="PSUM") as psl,
            tc.tile_pool(name="psr", bufs=2, space="PSUM") as psr,
        ):
            # ---- constants / per-core setup ----
            hp = tc.high_priority(offset=100000)
            hp.__enter__()
            u2b = cst.tile([128, 128], bf16)
            nc.sync.dma_start(u2b[:], u2b_d[:])
            onb = cst.tile([128, 128], bf16)
            nc.sync.dma_start(onb[:], onb_d[:])
            usr = cst.tile([128, 128], f32r)
            nc.sync.dma_start(usr[:], usr_d[:])
            onr = cst.tile([128, 128], f32r)
            nc.sync.dma_start(onr[:], onr_d[:])
            stc = cst.tile([128, 4, 8], f32r)
            nc.sync.dma_start(stc[:], stc_d[:])
            bgw = cst.tile([1, 8], f32r)
            nc.sync.dma_start(bgw[:], bgw_d[:])
            thr = cst.tile([128, 1], f32)
            nc.sync.dma_start(thr[:], thr_d[:])
            p64 = cst.tile([1, 128], bf16)
            nc.sync.dma_start(p64[:], p64_d[:])
            onerow = cst.tile([1, F], bf16)
            nc.vector.memset(onerow[:], 1.0)
            neg01 = cst.tile([128, 1], f32)
            nc.sync.dma_start(neg01[:], neg01_d[:])

            par0 = cst.tile([128, 8], f32)
            nc.sync.dma_start(par0[:], p_d[0:128, :])
            par1 = cst.tile([128, 8], f32)
            nc.sync.dma_start(par1[:], p_d[128:256, :])
            st3l = cst.tile([128, 8], f32r)
            nc.sync.dma_start(st3l[:], z8_d[:])
            st3h = cst.tile([128, 8], f32r)
            nc.sync.dma_start(st3h[:], z8_d[:])
            nc.vector.tensor_tensor(st3l[:, 4:5], par0[:, 2:3], par0[:, 3:4], MUL)
            nc.vector.tensor_tensor(st3h[:, 4:5], par1[:, 2:3], par1[:, 3:4], MUL)
            hp.__exit__(None, None, None)

            # ---- main loop over pixel tiles ----
            FC = F // 2  # psum chunk width
            for t in range(NT):
                px = bass.ts(t, F)

                At = io.tile([128, 2, F], f32, tag="A")
                Ct = io.tile([128, 2, 3, F], f32, tag="C")
                with tc.high_priority(offset=60):
                    nc.sync.dma_start(
                        At[:],
                        bass.AP(a_d, t * F, [[PIX, 128], [128 * PIX, 2], [1, F]]),
                    )
                    for h in range(2):
                        nc.sync.dma_start(
                            Ct[:, h],
                            bass.AP(
                                c_d, h * 128 * 3 * PIX + t * F,
                                [[3 * PIX, 128], [PIX, 3], [1, F]],
                            ),
                        )

                # msign = sign(A - 0.1) in {-1,+1}; mask algebra folded into
                # halved stationaries, per-row thresholds and +64 rank-1s.
                ms = wk.tile([128, 2, F], bf16, tag="ms")
                nc.scalar.activation(ms[:], At[:], Sign, bias=neg01[:])

                aet = wk.tile([128, 2, F], f32, tag="aet")
                lgt = wk.tile([128, 2, F], f32r, tag="lgt")
                Txt = wk.tile([128, 2, F], f32, tag="Txt")
                wt = wk.tile([128, 2, 1, F], f32r, tag="wt")
                Zt = wk.tile([128, 2, 3, F], f32r, tag="Zt")
                red = psr.tile([8, F], f32, tag="red")

                for c in range(2):
                    cs = bass.ts(c, FC)

                    q2 = psq.tile([128, 2, FC], f32, tag="qc")
                    nc.tensor.matmul(q2[:, 1, :], u2b[:], ms[:, 1, cs], start=True, stop=True)
                    nc.tensor.matmul(q2[:, 0, :], u2b[:], ms[:, 0, cs], start=True, stop=False)
                    nc.tensor.matmul(q2[:, 0, :], onb[:], ms[:, 1, cs], start=False, stop=False)
                    nc.tensor.matmul(q2[:, 0, :], p64[:], onerow[0:1, cs], start=False, stop=True)

                    # ae = (q <= thr) * A
                    nc.vector.scalar_tensor_tensor(
                        aet[:, :, cs], q2[:], thr[:], At[:, :, cs], LE, MUL
                    )
                    # lg = ln(1 - ae)
                    nc.scalar.activation(
                        lgt[:, :, cs], aet[:, :, cs], Ln, bias=1.0, scale=-1.0
                    )
                    # suffix log-sums
                    Lx = psl.tile([128, 2, FC], f32, tag="lxc")
                    nc.tensor.matmul(Lx[:, 1, :], usr[:], lgt[:, 1, cs], start=True, stop=True)
                    nc.tensor.matmul(Lx[:, 0, :], usr[:], lgt[:, 0, cs], start=True, stop=False)
                    nc.tensor.matmul(Lx[:, 0, :], onr[:], lgt[:, 1, cs], start=False, stop=True)

                    nc.scalar.activation(Txt[:, :, cs], Lx[:], Exp)

                    # w = ae * Texcl
                    nc.vector.tensor_tensor(
                        wt[:, :, 0, cs], aet[:, :, cs], Txt[:, :, cs], MUL
                    )
                    # Z = w (broadcast over channels) * color
                    nc.vector.tensor_tensor(
                        Zt[:, :, :, cs],
                        wt[:, :, :, cs].to_broadcast([128, 2, 3, FC]),
                        Ct[:, :, :, cs],
                        MUL,
                    )
                    # reductions: row0 sum(lg), rows 1-3 canvas rgb, row 4 den
                    nc.tensor.matmul(red[:, cs], stc[:, 0, :], Zt[:, 1, 0, cs], start=(c == 0), stop=False, skip_group_check=True)
                    nc.tensor.matmul(red[:, cs], stc[:, 0, :], Zt[:, 0, 0, cs], start=False, stop=False, skip_group_check=True)
                    nc.tensor.matmul(red[:, cs], stc[:, 1, :], Zt[:, 1, 1, cs], start=False, stop=False, skip_group_check=True)
                    nc.tensor.matmul(red[:, cs], stc[:, 1, :], Zt[:, 0, 1, cs], start=False, stop=False, skip_group_check=True)
                    nc.tensor.matmul(red[:, cs], stc[:, 2, :], Zt[:, 1, 2, cs], start=False, stop=False, skip_group_check=True)
                    nc.tensor.matmul(red[:, cs], stc[:, 2, :], Zt[:, 0, 2, cs], start=False, stop=False, skip_group_check=True)
                    nc.tensor.matmul(red[:, cs], st3h[:], wt[:, 1, 0, cs], start=False, stop=False, skip_group_check=True)
                    nc.tensor.matmul(red[:, cs], st3l[:], wt[:, 0, 0, cs], start=False, stop=False, skip_group_check=True)
                    nc.tensor.matmul(red[:, cs], stc[:, 3, :], lgt[:, 1, cs], start=False, stop=False, skip_group_check=True)
                    nc.tensor.matmul(red[:, cs], stc[:, 3, :], lgt[:, 0, cs], start=False, stop=False, skip_group_check=True)

                # background transmittance onto rows 1-4 via rank-1 matmul
                bg = wk.tile([1, F], f32r, tag="bg")
                nc.scalar.activation(bg[:], red[0:1, :], Exp)
                nc.tensor.matmul(red[:], bgw[:], bg[:], start=False, stop=True, skip_group_check=True)

                outt = wk.tile([8, F], f32, tag="outt")
                nc.scalar.copy(outt[:], red[:])
                nc.scalar.dma_start(out_d[:, px], outt[1:5, :])

    nc.compile()
    return nc




# ---------------- fast path: selection confined to the last NS strokes ------
NS = 32          # strokes kept (base NB..256); guarded exactly in kernel()
NB = N - NS      # 224
G = 4            # pixel groups packed along the partition dim
FF = 256         # pixels per group per fast tile
FT = G * FF      # pixels per fast tile (1024)
NTF = PIX // FT  # 8 tiles
PIX4 = PIX // G  # packed free extent (2048)


def _build_fast():
    _patch_act_tables()
    nc = bacc.Bacc("TRN2", target_bir_lowering=False, debug=False, num_devices=8)

    a_d = nc.dram_tensor("alpha32", [128, NTF, FF], f32, kind="ExternalInput")
    c_d = nc.dram_tensor("color32", [128, 3, NTF, FF], f32, kind="ExternalInput")
    p_d = nc.dram_tensor("par32", [NS, 8], f32, kind="ExternalInput")
    u2_d = nc.dram_tensor("U2BD", [128, 128], bf16, kind="ExternalInput")
    us_d = nc.dram_tensor("USBD", [128, 128], f32r, kind="ExternalInput")
    thr_d = nc.dram_tensor("THR32", [128, 1], f32, kind="ExternalInput")
    neg01_d = nc.dram_tensor("NEG01", [128, 1], f32, kind="ExternalInput")
    sel_d = nc.dram_tensor("SELS", [128, 4, 20], f32r, kind="ExternalInput")
    sden_d = nc.dram_tensor("SDEN0", [128, 20], f32r, kind="ExternalInput")
    bgw_d = nc.dram_tensor("BGW4", [4, 20], f32r, kind="ExternalInput")
    out_d = nc.dram_tensor("out", [4, PIX], f32, kind="ExternalOutput")

    Ln = mybir.ActivationFunctionType.Ln
    Exp = mybir.ActivationFunctionType.Exp
    Sign = mybir.ActivationFunctionType.Sign
    MUL = mybir.AluOpType.mult
    LE = mybir.AluOpType.is_le

    with tile.TileContext(nc) as tc:
        with (
            tc.tile_pool(name="cst", bufs=1) as cst,
            tc.tile_pool(name="io", bufs=6) as io,
            tc.tile_pool(name="wk", bufs=5) as wk,
            tc.tile_pool(name="psq", bufs=3, space="PSUM") as psq,
            tc.tile_pool(name="psl", bufs=3, space="PSUM") as psl,
            tc.tile_pool(name="psr", bufs=2, space="PSUM") as psr,
        ):
            hp = tc.high_priority(offset=100000)
            hp.__enter__()
            u2b = cst.tile([128, 128], bf16)
            nc.sync.dma_start(u2b[:], u2_d[:])
            usr = cst.tile([128, 128], f32r)
            nc.sync.dma_start(usr[:], us_d[:])
            thr = cst.tile([128, 1], f32)
            nc.sync.dma_start(thr[:], thr_d[:])
            neg01 = cst.tile([128, 1], f32)
            nc.sync.dma_start(neg01[:], neg01_d[:])
            sel = cst.tile([128, 4, 20], f32r)
            nc.sync.dma_start(sel[:], sel_d[:])
            sden = cst.tile([128, 20], f32r)
            nc.sync.dma_start(sden[:], sden_d[:])
            bgw = cst.tile([4, 20], f32r)
            nc.sync.dma_start(bgw[:], bgw_d[:])
            par = cst.tile([NS, 8], f32)
            nc.sync.dma_start(par[:], p_d[:])
            s32 = cst.tile([NS, 1], f32r)
            nc.vector.tensor_tensor(s32[:], par[:, 2:3], par[:, 3:4], MUL)
            warm = cst.tile([128, 1], f32)
            nc.scalar.activation(warm[:], neg01[:], Sign, bias=0.0)
            nc.scalar.activation(warm[:], neg01[:], Ln, bias=0.0, scale=-10.0)
            nc.scalar.activation(warm[:], warm[:], Exp)
            for g in range(G):
                nc.vector.tensor_copy(sden[32 * g : 32 * g + 32, 16 + g : 17 + g], s32[:])
            hp.__exit__(None, None, None)

            for t in range(NTF):
                base = t * FT
                At = io.tile([128, FF], f32, tag="A")
                Ct = io.tile([128, 3, FF], f32, tag="C")
                with tc.high_priority(offset=60):
                    nc.sync.dma_start(At[:], a_d[:, t, :])
                    nc.sync.dma_start(Ct[:], c_d[:, :, t, :])

                ms = wk.tile([128, FF], bf16, tag="ms")
                nc.scalar.activation(ms[:], At[:], Sign, bias=neg01[:])
                q2 = psq.tile([128, FF], f32, tag="q")
                nc.tensor.matmul(q2[:], u2b[:], ms[:], start=True, stop=True)
                aet = wk.tile([128, FF], f32, tag="aet")
                nc.vector.scalar_tensor_tensor(aet[:], q2[:], thr[:], At[:], LE, MUL)
                lgt = wk.tile([128, FF], f32r, tag="lgt")
                nc.scalar.activation(lgt[:], aet[:], Ln, bias=1.0, scale=-1.0)
                Lx = psl.tile([128, FF], f32, tag="Lx")
                nc.tensor.matmul(Lx[:], usr[:], lgt[:], start=True, stop=True)
                Txt = wk.tile([128, FF], f32, tag="Txt")
                nc.scalar.activation(Txt[:], Lx[:], Exp)
                wt = wk.tile([128, 1, FF], f32r, tag="wt")
                nc.vector.tensor_tensor(wt[:, 0, :], aet[:], Txt[:], MUL)
                Zt = wk.tile([128, 3, FF], f32r, tag="Zt")
                nc.vector.tensor_tensor(
                    Zt[:], wt[:].to_broadcast([128, 3, FF]), Ct[:], MUL
                )

                red = psr.tile([20, FF], f32, tag="red")
                nc.tensor.matmul(red[:], sel[:, 0, :], lgt[:], start=True, stop=False)
                nc.tensor.matmul(red[:], sel[:, 1, :], Zt[:, 0, :], start=False, stop=False)
                nc.tensor.matmul(red[:], sel[:, 2, :], Zt[:, 1, :], start=False, stop=False)
                nc.tensor.matmul(red[:], sel[:, 3, :], Zt[:, 2, :], start=False, stop=False)
                nc.tensor.matmul(red[:], sden[:], wt[:, 0, :], start=False, stop=False)
                bg = wk.tile([4, FF], f32r, tag="bg")
                nc.scalar.activation(bg[:], red[0:4, :], Exp)
                nc.tensor.matmul(red[:], bgw[:], bg[:], start=False, stop=True)

                outt = wk.tile([20, FF], f32, tag="outt")
                nc.scalar.copy(outt[:], red[:])
                nc.scalar.dma_start(
                    bass.AP(out_d, base, [[PIX, 4], [FF, 4], [1, FF]]),
                    outt[4:20, :],
                )

    nc.compile()
    return nc


def _fast_consts():
    if "fconsts" in _CACHED:
        return _CACHED["fconsts"]
    tri = np.tril(np.ones((NS, NS), np.float32), -1)
    u2h = 0.5 * tri - (BIG / 2) * np.eye(NS, dtype=np.float32)
    u2bd = np.zeros((128, 128), np.float32)
    usbd = np.zeros((128, 128), np.float32)
    for g in range(G):
        sl = slice(32 * g, 32 * g + 32)
        u2bd[sl, sl] = u2h
        usbd[sl, sl] = tri
    r32 = u2h.sum(axis=0)
    thr = np.tile(np.float32(THRESH) - r32, G).reshape(128, 1).astype(np.float32)
    sels = np.zeros((128, 4, 20), np.float32)
    bgw4 = np.zeros((4, 20), np.float32)
    for g in range(G):
        sl = slice(32 * g, 32 * g + 32)
        sels[sl, 0, g] = 1.0           # lgsum -> row g
        sels[sl, 1, 4 + g] = 1.0       # canvas r -> rows 4-7
        sels[sl, 2, 8 + g] = 1.0       # canvas g -> rows 8-11
        sels[sl, 3, 12 + g] = 1.0      # canvas b -> rows 12-15
        bgw4[g, 4 + g] = 1.0
        bgw4[g, 8 + g] = 1.0
        bgw4[g, 12 + g] = 1.0
        bgw4[g, 16 + g] = 1.0
    fc = {
        "U2BD": u2bd.astype(ml_dtypes.bfloat16),
        "USBD": usbd,
        "THR32": thr,
        "NEG01": np.full((128, 1), -np.float32(0.1), np.float32),
        "SELS": sels,
        "SDEN0": np.zeros((128, 20), np.float32),
        "BGW4": bgw4,
    }
    _CACHED["fconsts"] = fc
    return fc


def _get_program():
    if "nc" not in _CACHED:
        _CACHED["nc"] = _build_program()
    return _CACHED["nc"]


def _consts():
    if "consts" in _CACHED:
        return _CACHED["consts"]
    tri = np.tril(np.ones((128, 128), np.float32), -1)
    # halved: the matmul consumes msign in {-1,+1}; q = U2h@msign + r
    u2 = 0.5 * tri - (BIG / 2) * np.eye(128, dtype=np.float32)
    ones = np.ones((128, 128), np.float32)
    r = u2.sum(axis=0).astype(np.float32).reshape(128, 1)   # per-row offset
    thr = np.float32(THRESH) - r
    stc = np.zeros((128, 4, 8), np.float32)
    stc[:, 0, 1] = 1.0   # canvas r -> row 1
    stc[:, 1, 2] = 1.0   # canvas g -> row 2
    stc[:, 2, 3] = 1.0   # canvas b -> row 3
    stc[:, 3, 0] = 1.0   # sum(lg) -> row 0
    bgw = np.zeros((1, 8), np.float32)
    bgw[0, 1:5] = 1.0
    consts = {
        "U2B": u2.astype(ml_dtypes.bfloat16),
        "ONESB": (0.5 * ones).astype(ml_dtypes.bfloat16),
        "THR": thr,
        "P64": np.full((1, 128), 64.0, ml_dtypes.bfloat16),
        "NEG01": np.full((128, 1), -np.float32(0.1), np.float32),
        "USR": tri,
        "ONESR": ones,
        "STC": stc,
        "BGW": bgw,
        "Z8": np.zeros((128, 8), np.float32),
    }
    _CACHED["consts"] = consts
    return consts


def _make_in_maps(color_stroke, alpha, params):
    consts = _consts()
    in_maps = []
    for core in range(8):
        b, half = core // 2, core % 2
        r0 = half * (H // 2)
        a = np.ascontiguousarray(
            alpha[b, :, 0, r0 : r0 + H // 2, :].reshape(N, PIX)
        )
        c = np.ascontiguousarray(
            color_stroke[b, :, :, r0 : r0 + H // 2, :].reshape(N, 3, PIX)
        )
        p = np.ascontiguousarray(params[b])
        in_maps.append({"alpha": a, "color": c, "par": p, **consts})
    return in_maps


def kernel(color_stroke, alpha, params, _trace=False, _trace_kwargs=None):
    color_stroke = np.asarray(color_stroke, dtype=np.float32)
    alpha = np.asarray(alpha, dtype=np.float32)
    params = np.asarray(params, dtype=np.float32)

    # Guard: the per-pixel top-10 selection is provably confined to the last
    # NS strokes iff every pixel has >= TOPK visible strokes there.
    a_tail = alpha[:, NB:, 0]                       # [B, NS, H, W]
    cnt_min = (a_tail > np.float32(0.1)).sum(axis=1).min()
    fast = cnt_min >= 10

    if fast:
        if "ncf" not in _CACHED:
            _CACHED["ncf"] = _build_fast()
        nc = _CACHED["ncf"]
        fc = _fast_consts()
        in_maps = []
        for core in range(8):
            b, half = core // 2, core % 2
            r0 = half * (H // 2)
            a32 = alpha[b, NB:, 0, r0 : r0 + H // 2, :].reshape(NS, PIX)
            a = np.ascontiguousarray(
                a32.reshape(NS, NTF, G, FF).transpose(2, 0, 1, 3).reshape(128, NTF, FF)
            )
            c32 = color_stroke[b, NB:, :, r0 : r0 + H // 2, :].reshape(NS, 3, PIX)
            c = np.ascontiguousarray(
                c32.reshape(NS, 3, NTF, G, FF)
                .transpose(3, 0, 1, 2, 4)
                .reshape(128, 3, NTF, FF)
            )
            p = np.ascontiguousarray(params[b, NB:])
            in_maps.append({"alpha32": a, "color32": c, "par32": p, **fc})
    else:
        nc = _get_program()
        in_maps = _make_in_maps(color_stroke, alpha, params)

    res = run_bass_kernel_spmd(
        nc, in_maps, list(range(8)), trace=_trace, **(_trace_kwargs or {})
    )
    _CACHED["last_result"] = res

    canvas = np.empty((B, 3, H, W), np.float32)
    den = np.empty((B, 1, H, W), np.float32)
    for core in range(8):
        b, half = core // 2, core % 2
        r0 = half * (H // 2)
        o = res.results[core]["out"]
        canvas[b, :, r0 : r0 + H // 2, :] = o[0:3].reshape(3, H // 2, W)
        den[b, 0, r0 : r0 + H // 2, :] = o[3].reshape(H // 2, W)
    return canvas, den
